# revision 32
# baseline (speedup 1.0000x reference)
"""Multi-head attention (B=4, S=1024, D=1024, H=16, DH=64) on 8 trn2 cores.

Tensor-parallel over heads: core c owns heads {2c, 2c+1}; each core runs
8 independent attention units (4 batches x 2 heads).  Per-head projections
only read a 64-channel slice of the input, so each core receives just its
2x64-channel slice, pre-transposed to [d, s] with a ones-row appended
(E1 = 66: row 64 is the ones row, 65 zero pad).

Math per unit (b, h).  The Wk^T.Wq product is folded on the host
(G^T = Wq~ @ Wk~^T, biases/scale included via the ones-row), so only one
projection feeds the scores:
  y[c,s]    = G^T.T @ xTe           (one 66x66 "projection" replaces q,k)
  scT[t,s]  = xTe.T @ y             (= q.k scores, transposed: t on parts)
  v[t,e']   = xbT.T @ WvTe2         (bf16 inputs: 1 cyc/row; col 64 == 1
                                     -> Z column of out2)
  expT      = exp(scT) -> bf16      (no max-subtraction: |scores| <= ~10)
  out2[s,e']= sum_t expT[t,s] v[t,e']   (transposed PV: s on partitions,
                                     col 64 = Z[s]; per s-block 8
                                     accumulating 66-row bf16 matmuls)
  out[s,e]  = out2[s,e] / Z[s]      (batched 4-way reciprocal + per-sb
                                     tensor_scalar multiply)

Scores stream through a ring of THREE 2-bank PSUM generation tiles
([128, 1024] f32) in 512-col pieces (256-col for the first two, which
shortens the fill critical path to the first exp).  exp is split
across TWO engines: ACT chunks use the native Exp; every 4th chunk
(DVE_CHUNKS, 25% of columns) runs on the Vector engine via the
Schraudolph exponent-bit trick -- one tensor_scalar
i16 = round(x * 128/ln2 + B) written through a bf16 bitcast gives
2^(x*log2e) ~ exp(x) with ~3% max element error; since only a quarter
of the keys in each softmax use the approximation the end-to-end
output error stays ~9e-3, under the 2e-2 gate.  The ring depth of 3
is what lets an ACT exp and a DVE exp overlap: pieces of gen g only
wait the exp of gen g-2's chunk.  Engine busy lands near ACT ~50us /
PE ~47us / DVE ~44us instead of the ACT-bound 64us of the
single-engine version (73.3us -> 62.8us end to end).

Pieces are ordered s-major within each unit (s-half, then t-block), so
PV + normalize + store fire per HALF-unit (4 s-blocks, one shared
1-bank psum tile with 4 sequential accumulation groups) ~3 chunks
after the half's scores finish; one strided 4-Z-column reciprocal and
one broadcast tensor_mul normalize 256 output columns per burst.  The
V projection runs on a host-supplied bf16 copy of the input so its
66-col matmuls go 1 cyc/row (f32r pays 4x below 256 cols).  The last
half-unit is staged: sbl 0,1 park open accumulation groups in the two
psM banks (after chunk LAST-3's exp), sbl 2,3 in the retired ring
tile chunk LAST-2 read, so after the final exp only 8 matmuls, 4
normalizes (split ACT/DVE) and 2 narrow stores remain.  A dummy
2-element exp at kernel start pulls the 1.3us activation-table load
into the DMA fill window.
"""

import numpy as np

D = 1024
H = 16
DH = 64
B = 4
S = 1024
NCORES = 8
HPC = H // NCORES  # heads per core = 2
E1 = DH + 2  # 66: ones-row at 64, zero pad at 65
NT = S // 128  # 8 t blocks
NU = B * HPC  # 8 units per core
NHU = NU * 2  # 16 half-units (unit x s-half)
SCALE = 1.0 / np.sqrt(DH)

GEN = 1024  # scores generation: one 2-bank psum tile, in columns
RING = 3  # generations in flight
CHUNKS = [256, 256, 512] + [1024] * 63
N_CH = len(CHUNKS)
LAST = N_CH - 1
assert sum(CHUNKS) == NU * NT * S  # 65536 scores columns per core
CSTART = np.concatenate([[0], np.cumsum(CHUNKS)])
assert all(
    CSTART[i] // GEN == (CSTART[i] + CHUNKS[i] - 1) // GEN
    for i in range(N_CH)
)
# scores pieces: 256-col at the very start (shorter fill critical path),
# 512-col steady state
PIECES = [(0, 256), (256, 256)] + [(512 * k, 512) for k in range(1, 128)]

# chunks whose exp runs on the Vector engine (Schraudolph) instead of ACT.
# The half-unit completion chunks sit at c % 4 == 1 (PV/normalize bursts
# land on DVE right after them), so the DVE slots sit at c % 4 == 3 --
# except the last slot moves to the FINAL chunk, which lets ACT retire its
# stream early while the drain overlaps the final DVE exp.
DVE_CHUNKS = frozenset(range(3, 60, 4)) | {LAST}
A_EXP = 128.0 / np.log(2.0)
B_EXP = 16250.5  # 127*128 - c_opt(~5.5), tuned for hw round-to-nearest

_CACHE = {}


def _chunk_of(g):
    """Chunk index and in-chunk offset for global scores column g."""
    c = int(np.searchsorted(CSTART, g, side="right")) - 1
    return c, g - int(CSTART[c])


def _col(u, sh, tb, sb4=0):
    """Global scores column of unit u, s-half sh, t-block tb, s-subblock."""
    return u * 8192 + sh * 4096 + tb * 512 + sb4 * 128


# chunk whose exp completes each half-unit's scores
C_END_H = [_chunk_of(4096 * (hu + 1) - 1)[0] for hu in range(NHU)]


def _split_sync_waits(nc, limit=1):
    """Walrus in this toolchain rejects instructions carrying more than one
    sync-wait; peel extra waits onto wait-only EventSemaphore ops inserted
    just before, on the same engine queue (engine streams are in-order)."""
    import concourse.mybir as mybir

    n = 0
    for bb in nc.main_func.blocks:
        out = []
        for ins in bb.instructions:
            si = ins.sync_info
            if si is not None and len(si.on_wait) > limit:
                waits = list(si.on_wait)
                for w in waits[:-limit]:
                    ev = mybir.InstEventSemaphore(
                        name=f"WSPLIT-{n}", ins=[], outs=[]
                    )
                    n += 1
                    ev.engine = ins.engine
                    ev.sync_info = mybir.SyncInfo(on_wait=[w], on_update=[])
                    out.append(ev)
                ins.sync_info = mybir.SyncInfo(
                    on_wait=waits[-limit:], on_update=list(si.on_update)
                )
            out.append(ins)
        bb.instructions = out
    return n


def _build_bass(split=True):
    import concourse.bass as bass
    import concourse.mybir as mybir
    import concourse.tile as tile

    f32 = mybir.dt.float32
    f32r = mybir.dt.float32r
    bf16 = mybir.dt.bfloat16
    i16 = mybir.dt.int16
    nc = bass.Bass()

    xTe_d = nc.declare_dram_parameter("xTe", [B, HPC, E1, S], f32r, isOutput=False)
    xb_d = nc.declare_dram_parameter("xbTe", [B, HPC, E1, S], bf16, isOutput=False)
    gt_d = nc.declare_dram_parameter("GT", [E1, HPC * E1], f32r, isOutput=False)
    wv_d = nc.declare_dram_parameter("WvTe2", [E1, HPC * E1], bf16, isOutput=False)
    # out[b, j, p, blk*64 + e] == attention(b, s=blk*128+p, head j)[e]
    out_d = nc.declare_dram_parameter("out", [B, HPC, 128, 512], f32, isOutput=True)

    with tile.TileContext(nc) as tc:
        with (
            tc.tile_pool(name="const", bufs=1) as constp,
            tc.tile_pool(name="sb", bufs=2) as sbp,
            tc.tile_pool(name="expp", bufs=2) as expp,
            tc.tile_pool(name="psR", bufs=1, space="PSUM") as psR,
            tc.tile_pool(name="psM", bufs=2, space="PSUM") as psM,
        ):
            # dummy activation: pulls the exp table load into the DMA fill
            dummy = constp.tile([1, 4], f32, name="dummy")
            nc.gpsimd.memset(dummy[:], 0.0)
            nc.scalar.activation(
                dummy[:, 2:4], dummy[:, 0:2], mybir.ActivationFunctionType.Exp
            )

            gt_sb = constp.tile([E1, HPC * E1], f32r)
            wv_sb = constp.tile([E1, HPC * E1], bf16)
            nc.gpsimd.dma_start(gt_sb[:], gt_d[:])
            nc.gpsimd.dma_start(wv_sb[:], wv_d[:])

            gens = {}  # generation idx -> ring scores tile

            def new_gen(i):
                for k in range(max(gens, default=-1) + 1, i + 1):
                    gens[k] = psR.tile([128, GEN], f32, tag="sc", bufs=RING,
                                       name=f"sc_{k}")
                return gens[i]

            # PE warmup: a 2-col matmul at t~0 so the fill-phase projection
            # matmuls run at ramped pstate instead of cold
            nc.tensor.matmul(
                new_gen(0)[:2, 0:2], dummy[:, 0:2], dummy[:, 2:4],
                start=True, stop=True,
            )

            units = [(b, j) for b in range(B) for j in range(HPC)]

            xts = {}
            xbs = {}

            def fetch_xt(b, j):
                if (b, j) not in xts:
                    for jj in range(HPC):
                        xts[(b, jj)] = sbp.tile(
                            [E1, S], f32r, tag="xt", bufs=4,
                            name=f"xt_{b}_{jj}",
                        )
                        xbs[(b, jj)] = sbp.tile(
                            [E1, S], bf16, tag="xb", bufs=4,
                            name=f"xb_{b}_{jj}",
                        )
                    for jj in range(HPC):  # j-major: head 0 complete first
                        # finer first transfer for unit 0: the first y chunk
                        # and scores piece only need 256 columns
                        spans = ([(0, 256), (256, 256), (512, 512)]
                                 if b == 0 and jj == 0 else
                                 [(0, 512), (512, 512)])
                        for s0, w in spans:
                            nc.sync.dma_start(
                                xts[(b, jj)][:, s0:s0 + w],
                                xTe_d[b, jj, :, s0:s0 + w],
                            )
                    for jj in range(HPC):
                        nc.sync.dma_start(xbs[(b, jj)][:], xb_d[b, jj])
                return xts[(b, j)]

            def y_chunks(b, j, fine=False):
                """Scores projection y = G^T.T @ x as filler chunks."""
                xt = fetch_xt(b, j)
                yT = sbp.tile([E1, S], f32r, tag="yT", bufs=3, name=f"y_{b}_{j}")
                chunks = []
                spans = ([(0, 256), (256, 256), (512, 512)] if fine
                         else [(0, 512), (512, 512)])
                for s0, w in spans:
                    def chunk(s0=s0, w=w):
                        y_ps = psM.tile(
                            [128, 512], f32, tag="m", bufs=2, name="y_ps"
                        )
                        nc.tensor.matmul(
                            y_ps[:E1, :w],
                            gt_sb[:, j * E1:(j + 1) * E1],
                            xt[:, s0:s0 + w],
                            start=True, stop=True,
                        )
                        nc.vector.tensor_copy(yT[:, s0:s0 + w], y_ps[:E1, :w])
                    chunks.append(chunk)
                return yT, chunks

            def v_chunks(b, j):
                """v projection as 2 filler chunks (4 bf16 MMs + copy each).

                bf16 x bf16 matmuls run 1 cyc/row even at 66-col outputs
                (f32r would pay 4x below 256 cols), fed by the host-packed
                bf16 copy of the input slice.
                """
                fetch_xt(b, j)
                xb = xbs[(b, j)]
                v_sb = sbp.tile(
                    [128, NT * E1], bf16, tag="v", bufs=4, name=f"v_{b}_{j}"
                )
                chunks = []
                for half in range(2):
                    def chunk(half=half):
                        v_ps = psM.tile(
                            [128, 512], f32, tag="m", bufs=2, name="v_ps"
                        )
                        for q in range(4):
                            tb = half * 4 + q
                            nc.tensor.matmul(
                                v_ps[:, q * E1:(q + 1) * E1],
                                xb[:, tb * 128:(tb + 1) * 128],
                                wv_sb[:, j * E1:(j + 1) * E1],
                                start=True, stop=True,
                            )
                        nc.vector.tensor_copy(
                            v_sb[:, half * 4 * E1:(half + 1) * 4 * E1],
                            v_ps[:, :4 * E1],
                        )
                    chunks.append(chunk)
                return v_sb, chunks

            chunk_exp = {}  # chunk idx -> expT tile

            def emit_exp(c):
                """exp of completed chunk c: ACT Exp, or DVE Schraudolph."""
                csz = CHUNKS[c]
                cp = int(CSTART[c]) % GEN
                src = gens[int(CSTART[c]) // GEN]
                expT = expp.tile(
                    [128, GEN], bf16, tag="expT", bufs=16, name="expT"
                )
                if c in DVE_CHUNKS:
                    nc.vector.tensor_scalar(
                        expT[:, :csz].bitcast(i16),
                        src[:, cp:cp + csz],
                        float(A_EXP),
                        float(B_EXP),
                        mybir.AluOpType.mult,
                        mybir.AluOpType.add,
                    )
                else:
                    nc.scalar.activation(
                        expT[:, :csz], src[:, cp:cp + csz],
                        mybir.ActivationFunctionType.Exp,
                    )
                chunk_exp[c] = expT

            o_sbs = {}

            def get_o(u):
                if u not in o_sbs:
                    b, j = units[u]
                    o_sbs[u] = sbp.tile([128, 512], f32, tag="o", bufs=2,
                                        name=f"o_{b}_{j}")
                return o_sbs[u]

            def pv_mms(u, sh, sbl, out2, v_sb, tbs, first):
                for tb in tbs:
                    c, off = _chunk_of(_col(u, sh, tb, sbl))
                    nc.tensor.matmul(
                        out2,
                        chunk_exp[c][:, off:off + 128],
                        v_sb[:, tb * E1:(tb + 1) * E1],
                        start=(tb == tbs[0] and first),
                        stop=(tb == NT - 1),
                    )

            def scale_sb(u, sh, sbl, out2_64, invz_col, eng="dve"):
                o_sl = get_o(u)[:, (sh * 4 + sbl) * DH:(sh * 4 + sbl + 1) * DH]
                if eng == "act":
                    nc.scalar.activation(
                        o_sl, out2_64,
                        mybir.ActivationFunctionType.Copy,
                        scale=invz_col,
                    )
                else:
                    nc.vector.tensor_scalar_mul(o_sl, out2_64, invz_col)

            def pv_burst(hu, v_sb):
                """PV + normalize + store for half-unit hu (4 s-blocks).

                All four out2 regions live in ONE 1-bank psum tile (the
                accumulation groups run sequentially), so a single strided
                reciprocal covers the four Z columns."""
                u, sh = hu // 2, hu % 2
                shared = {}
                chunks = []
                for sbl in range(4):
                    def chunk(sbl=sbl):
                        if "out2" not in shared:
                            shared["out2"] = psM.tile(
                                [128, 512], f32, tag="m", bufs=2,
                                name=f"pv_{hu}",
                            )
                        out2 = shared["out2"]
                        pv_mms(u, sh, sbl, out2[:, sbl * E1:sbl * E1 + E1],
                               v_sb, list(range(NT)), True)
                    chunks.append(chunk)

                def norms():
                    out2 = shared["out2"]
                    invz = sbp.tile([128, 4], f32, tag="invz", bufs=8,
                                    name="invz")
                    nc.vector.reciprocal(
                        invz[:], out2[:, DH:3 * E1 + DH + 1:E1]
                    )
                    # one 256-col multiply for all 4 s-blocks: the strided
                    # out2 view picks the 64 value cols of each block and
                    # invz broadcasts along the inner dim.
                    o = get_o(u)[:, sh * 256:(sh + 1) * 256]
                    nc.vector.tensor_mul(
                        o.rearrange("p (b e) -> p b e", e=DH),
                        out2[:, :4 * E1].rearrange(
                            "p (b e) -> p b e", e=E1)[:, :, :DH],
                        invz[:].rearrange("p (b e) -> p b e", e=1)
                        .broadcast_to([128, 4, DH]),
                    )
                chunks.append(norms)

                def store():
                    nc.sync.dma_start(
                        out_d[units[u][0], units[u][1]][:, sh * 256:(sh + 1) * 256],
                        get_o(u)[:, sh * 256:(sh + 1) * 256],
                    )
                chunks.append(store)
                return chunks

            # --- last half-unit (hu = 15): staged so almost nothing trails
            # the final exp.  sbl 0,1 park open accumulation groups in the
            # two psM banks (after chunk LAST-3's exp); sbl 2,3 in the two
            # banks of the retired gen tile that chunk LAST-2 read.  After
            # the final exp: 2 matmuls per s-block, 4 normalizes
            # (alternating DVE / ACT), 2 narrow stores.
            last_state = {}

            def last_wave(ceil_c, allowed=(0, 1, 2, 3)):
                u, sh = NU - 1, 1
                v_sb = unit_io[u][1]
                st = last_state
                if "done" not in st:
                    st["done"] = [0] * 4
                    st["out2"] = {}
                    st["started"] = [False] * 4
                chunks = []
                for sbl in allowed:
                    tbs = [tb for tb in range(NT)
                           if st["done"][sbl] <= tb
                           and _chunk_of(_col(u, sh, tb, sbl))[0] <= ceil_c]
                    if not tbs:
                        continue
                    if sbl not in st["out2"]:
                        if sbl < 2:
                            t = psM.tile([128, 512], f32, tag="m", bufs=2,
                                         name=f"lpv_{sbl}")
                            st["out2"][sbl] = t[:, :E1]
                        else:
                            # the tile chunk LAST-2 read is retired once that
                            # exp ran (the final chunk lives in a different
                            # ring slot) -- park sbl 2,3 in its two banks.
                            gi = int(CSTART[LAST - 2]) // GEN
                            off = (sbl - 2) * 512
                            st["out2"][sbl] = new_gen(gi)[:, off:off + E1]
                    out2 = st["out2"][sbl]
                    first = not st["started"][sbl]
                    st["started"][sbl] = True
                    st["done"][sbl] = tbs[-1] + 1

                    def ch(sbl=sbl, tbs=tbs, out2=out2, first=first):
                        pv_mms(u, sh, sbl, out2, v_sb, tbs, first)
                    chunks.append(ch)
                return chunks

            def last_fin():
                u, sh = NU - 1, 1
                b, j = units[u]
                chunks = list(last_wave(LAST))

                def norms_a():
                    st = last_state
                    invz = sbp.tile([128, 2], f32, tag="invz", bufs=8,
                                    name="invz")
                    nc.vector.reciprocal(invz[:, 0:1], st["out2"][0][:, DH:DH + 1])
                    nc.vector.reciprocal(invz[:, 1:2], st["out2"][1][:, DH:DH + 1])
                    scale_sb(u, sh, 0, st["out2"][0][:, :DH], invz[:, 0:1], "dve")
                    scale_sb(u, sh, 1, st["out2"][1][:, :DH], invz[:, 1:2], "act")
                chunks.append(norms_a)

                def norms_b():
                    st = last_state
                    invz = sbp.tile([128, 2], f32, tag="invz", bufs=8,
                                    name="invz")
                    # sbl 2,3 live in the same ring tile: one strided recip
                    g61 = new_gen(int(CSTART[LAST - 2]) // GEN)
                    nc.vector.reciprocal(invz[:], g61[:, DH:DH + 513:512])
                    scale_sb(u, sh, 2, st["out2"][2][:, :DH], invz[:, 0:1], "dve")
                    scale_sb(u, sh, 3, st["out2"][3][:, :DH], invz[:, 1:2], "act")
                chunks.append(norms_b)

                def store_a():
                    nc.sync.dma_start(
                        out_d[b, j][:, 256:384], get_o(u)[:, 256:384]
                    )
                chunks.append(store_a)

                def store_b():
                    nc.sync.dma_start(
                        out_d[b, j][:, 384:512], get_o(u)[:, 384:512]
                    )
                chunks.append(store_b)
                return chunks

            # Software pipeline: scores pieces stream through the psum ring
            # in 512-col steps (s-major within each unit); exp fires per
            # chunk on ACT or DVE; projection chunks of the next unit and
            # PV/normalize/store chunks of completed half-units interleave
            # as fillers.
            from collections import deque

            fillers = deque()
            unit_io = {}

            def unit_inputs(u):
                b, j = units[u]
                yT, ychunks = y_chunks(b, j, fine=(u == 0))
                for c in ychunks:
                    fillers.append(c)
                v_sb, vchunks = v_chunks(b, j)
                for c in vchunks:
                    fillers.append(c)
                unit_io[u] = (yT, v_sb)

            unit_inputs(0)
            u0_chunks = list(fillers)
            fillers.clear()
            for p, (g, w) in enumerate(PIECES):  # scores pieces, s-major
                u = g // 8192
                sh = (g % 8192) // 4096
                tb = (g % 4096) // 512
                so = sh * 512 + (g % 512)  # s-offset within the unit's half
                if p < len(u0_chunks):
                    u0_chunks[p]()  # y chunks before their pieces, then v
                if g % 8192 == 2048 and u + 1 < NU:
                    unit_inputs(u + 1)
                yT, _ = unit_io[u]
                if g % GEN == 0 and g // GEN not in gens:
                    new_gen(g // GEN)
                rp = g % GEN
                nc.tensor.matmul(
                    gens[g // GEN][:, rp:rp + w],
                    fetch_xt(*units[u])[:, tb * 128:(tb + 1) * 128],
                    yT[:, so:so + w],
                    start=True, stop=True,
                )
                c, _ = _chunk_of(g)
                if g + w == int(CSTART[c + 1]):  # chunk complete -> exp
                    emit_exp(c)
                    if c == LAST - 3:
                        # sbl 2,3 park in the tile chunk LAST-2 still reads
                        for ch in last_wave(c, allowed=(0, 1)):
                            fillers.append(ch)
                    elif c in (LAST - 2, LAST - 1):
                        for ch in last_wave(c):
                            fillers.append(ch)
                    elif c == LAST:
                        for ch in last_fin():
                            fillers.append(ch)
                    for hu in range(NHU - 1):
                        if C_END_H[hu] == c:
                            for ch in pv_burst(hu, unit_io[hu // 2][1]):
                                fillers.append(ch)
                for _ in range(2):
                    if fillers:
                        fillers.popleft()()
            while fillers:
                fillers.popleft()()
    if split:
        _split_sync_waits(nc)
    return nc


def _prep_inputs(sequences, Wq, Wk, Wv, bq, bk, bv):
    """Host-side packing: per-core input maps."""
    import ml_dtypes

    sequences = np.ascontiguousarray(np.asarray(sequences, dtype=np.float32))
    Wq = np.asarray(Wq, np.float32)
    Wk = np.asarray(Wk, np.float32)
    Wv = np.asarray(Wv, np.float32)
    bq = np.asarray(bq, np.float32)
    bk = np.asarray(bk, np.float32)
    bv = np.asarray(bv, np.float32)

    # [B, S, H, DH] -> [H, B, DH, S] transposed slices
    xT = np.ascontiguousarray(
        sequences.reshape(B, S, H, DH).transpose(2, 0, 3, 1)
    )  # [H, B, DH, S]

    in_maps = []
    for c in range(NCORES):
        heads = [HPC * c + j for j in range(HPC)]
        xTe = np.zeros((B, HPC, E1, S), np.float32)
        xTe[:, :, DH, :] = 1.0
        for j, h in enumerate(heads):
            xTe[:, j, :DH, :] = xT[h]
        gt = np.zeros((E1, HPC, E1), np.float32)
        wv = np.zeros((E1, HPC, E1), np.float32)
        for j, h in enumerate(heads):
            wq = np.zeros((E1, DH), np.float32)  # x~ -> q, scale folded
            wq[:DH] = Wq[h].T * SCALE
            wq[DH] = bq[h] * SCALE
            wk = np.zeros((E1, DH), np.float32)  # x~ -> k
            wk[:DH] = Wk[h].T
            wk[DH] = bk[h]
            # scores = k.q = x~^T (Wk~ Wq~^T) x~; lhsT of the y-projection
            # is the transpose: G^T = Wq~ @ Wk~^T
            gt[:, j, :] = wq @ wk.T
            wv[:DH, j, :DH] = Wv[h].T
            wv[DH, j, :DH] = bv[h]
            wv[DH, j, DH] = 1.0  # ones column -> Z column of out2
        in_maps.append({
            "xTe": xTe,
            "xbTe": xTe.astype(ml_dtypes.bfloat16),
            "GT": gt.reshape(E1, HPC * E1),
            "WvTe2": wv.reshape(E1, HPC * E1).astype(ml_dtypes.bfloat16),
        })
    return in_maps


def get_nc():
    if "nc" not in _CACHE:
        _CACHE["nc"] = _build_bass()
    return _CACHE["nc"]


def kernel(sequences, Wq, Wk, Wv, bq, bk, bv):
    from concourse.bass_utils import run_bass_kernel_spmd

    nc = get_nc()
    in_maps = _prep_inputs(sequences, Wq, Wk, Wv, bq, bk, bv)
    res = run_bass_kernel_spmd(nc, in_maps, list(range(NCORES)))
    full = np.empty((B, S, D), np.float32)
    for c in range(NCORES):
        # out[b, j, p, blk*64+e] -> full[b, blk*128+p, (2c+j)*64+e]
        arr = res.results[c]["out"].reshape(B, HPC, 128, NT, DH)
        full[:, :, c * HPC * DH:(c + 1) * HPC * DH] = (
            arr.transpose(0, 3, 2, 1, 4).reshape(B, S, HPC * DH)
        )
    return full


# revision 34
# speedup vs baseline: 1.0009x; 1.0009x over previous
"""Multi-head attention (B=4, S=1024, D=1024, H=16, DH=64) on 8 trn2 cores.

Tensor-parallel over heads: core c owns heads {2c, 2c+1}; each core runs
8 independent attention units (4 batches x 2 heads).  Per-head projections
only read a 64-channel slice of the input, so each core receives just its
2x64-channel slice, pre-transposed to [d, s] with a ones-row appended
(E1 = 66: row 64 is the ones row, 65 zero pad).

Math per unit (b, h).  The Wk^T.Wq product is folded on the host
(G^T = Wq~ @ Wk~^T, biases/scale included via the ones-row), so only one
projection feeds the scores:
  y[c,s]    = G^T.T @ xTe           (one 66x66 "projection" replaces q,k)
  scT[t,s]  = xTe.T @ y             (= q.k scores, transposed: t on parts)
  v[t,e']   = xbT.T @ WvTe2         (bf16 inputs: 1 cyc/row; col 64 == 1
                                     -> Z column of out2)
  expT      = exp(scT) -> bf16      (no max-subtraction: |scores| <= ~10)
  out2[s,e']= sum_t expT[t,s] v[t,e']   (transposed PV: s on partitions,
                                     col 64 = Z[s]; per s-block 8
                                     accumulating 66-row bf16 matmuls)
  out[s,e]  = out2[s,e] / Z[s]      (batched 4-way reciprocal + per-sb
                                     tensor_scalar multiply)

Scores stream through a ring of THREE 2-bank PSUM generation tiles
([128, 1024] f32) in 512-col pieces.  exp is split
across TWO engines: ACT chunks use the native Exp; every 4th chunk
(DVE_CHUNKS, 25% of columns) runs on the Vector engine via the
Schraudolph exponent-bit trick -- one tensor_scalar
i16 = round(x * 128/ln2 + B) written through a bf16 bitcast gives
2^(x*log2e) ~ exp(x) with ~3% max element error; since only a quarter
of the keys in each softmax use the approximation the end-to-end
output error stays ~9e-3, under the 2e-2 gate.  The ring depth of 3
is what lets an ACT exp and a DVE exp overlap: pieces of gen g only
wait the exp of gen g-2's chunk.  Engine busy lands near ACT ~50us /
PE ~47us / DVE ~44us instead of the ACT-bound 64us of the
single-engine version (73.3us -> 62.8us end to end).

Pieces are ordered s-major within each unit (s-half, then t-block), so
PV + normalize + store fire per HALF-unit (4 s-blocks, one shared
1-bank psum tile with 4 sequential accumulation groups) ~3 chunks
after the half's scores finish; one strided 4-Z-column reciprocal and
one broadcast tensor_mul normalize 256 output columns per burst.  The
V projection runs on a host-supplied bf16 copy of the input so its
66-col matmuls go 1 cyc/row (f32r pays 4x below 256 cols).  The last
half-unit is staged: sbl 0,1 park open accumulation groups in the two
psM banks (after chunk LAST-3's exp), sbl 2,3 in the retired ring
tile chunk LAST-2 read, so after the final exp only 8 matmuls, 4
normalizes (split ACT/DVE) and 2 narrow stores remain.  A dummy
2-element exp at kernel start pulls the 1.3us activation-table load
into the DMA fill window.
"""

import numpy as np

D = 1024
H = 16
DH = 64
B = 4
S = 1024
NCORES = 8
HPC = H // NCORES  # heads per core = 2
E1 = DH + 2  # 66: ones-row at 64, zero pad at 65
NT = S // 128  # 8 t blocks
NU = B * HPC  # 8 units per core
NHU = NU * 2  # 16 half-units (unit x s-half)
SCALE = 1.0 / np.sqrt(DH)

GEN = 1024  # scores generation: one 2-bank psum tile, in columns
RING = 3  # generations in flight
CHUNKS = [512, 512] + [1024] * 63
N_CH = len(CHUNKS)
LAST = N_CH - 1
assert sum(CHUNKS) == NU * NT * S  # 65536 scores columns per core
CSTART = np.concatenate([[0], np.cumsum(CHUNKS)])
assert all(
    CSTART[i] // GEN == (CSTART[i] + CHUNKS[i] - 1) // GEN
    for i in range(N_CH)
)
# scores pieces: uniform 512 columns (with the PE warmup, finer fill
# pieces no longer pay for their extra instruction overhead)
PIECES = [(512 * k, 512) for k in range(0, 128)]

# chunks whose exp runs on the Vector engine (Schraudolph) instead of ACT.
# The half-unit completion chunks sit at c % 4 == 1 (PV/normalize bursts
# land on DVE right after them), so the DVE slots sit at c % 4 == 3 --
# except the last slot moves to the FINAL chunk, which lets ACT retire its
# stream early while the drain overlaps the final DVE exp.
DVE_CHUNKS = frozenset(range(2, 59, 4)) | {LAST}
A_EXP = 128.0 / np.log(2.0)
B_EXP = 16250.5  # 127*128 - c_opt(~5.5), tuned for hw round-to-nearest

_CACHE = {}


def _chunk_of(g):
    """Chunk index and in-chunk offset for global scores column g."""
    c = int(np.searchsorted(CSTART, g, side="right")) - 1
    return c, g - int(CSTART[c])


def _col(u, sh, tb, sb4=0):
    """Global scores column of unit u, s-half sh, t-block tb, s-subblock."""
    return u * 8192 + sh * 4096 + tb * 512 + sb4 * 128


# chunk whose exp completes each half-unit's scores
C_END_H = [_chunk_of(4096 * (hu + 1) - 1)[0] for hu in range(NHU)]


def _split_sync_waits(nc, limit=1):
    """Walrus in this toolchain rejects instructions carrying more than one
    sync-wait; peel extra waits onto wait-only EventSemaphore ops inserted
    just before, on the same engine queue (engine streams are in-order)."""
    import concourse.mybir as mybir

    n = 0
    for bb in nc.main_func.blocks:
        out = []
        for ins in bb.instructions:
            si = ins.sync_info
            if si is not None and len(si.on_wait) > limit:
                waits = list(si.on_wait)
                for w in waits[:-limit]:
                    ev = mybir.InstEventSemaphore(
                        name=f"WSPLIT-{n}", ins=[], outs=[]
                    )
                    n += 1
                    ev.engine = ins.engine
                    ev.sync_info = mybir.SyncInfo(on_wait=[w], on_update=[])
                    out.append(ev)
                ins.sync_info = mybir.SyncInfo(
                    on_wait=waits[-limit:], on_update=list(si.on_update)
                )
            out.append(ins)
        bb.instructions = out
    return n


def _build_bass(split=True):
    import concourse.bass as bass
    import concourse.mybir as mybir
    import concourse.tile as tile

    f32 = mybir.dt.float32
    f32r = mybir.dt.float32r
    bf16 = mybir.dt.bfloat16
    i16 = mybir.dt.int16
    nc = bass.Bass()

    xTe_d = nc.declare_dram_parameter("xTe", [B, HPC, E1, S], f32r, isOutput=False)
    xb_d = nc.declare_dram_parameter("xbTe", [B, HPC, E1, S], bf16, isOutput=False)
    gt_d = nc.declare_dram_parameter("GT", [E1, HPC * E1], f32r, isOutput=False)
    wv_d = nc.declare_dram_parameter("WvTe2", [E1, HPC * E1], bf16, isOutput=False)
    # out[b, j, p, blk*64 + e] == attention(b, s=blk*128+p, head j)[e]
    out_d = nc.declare_dram_parameter("out", [B, HPC, 128, 512], f32, isOutput=True)

    with tile.TileContext(nc) as tc:
        with (
            tc.tile_pool(name="const", bufs=1) as constp,
            tc.tile_pool(name="sb", bufs=2) as sbp,
            tc.tile_pool(name="expp", bufs=2) as expp,
            tc.tile_pool(name="psR", bufs=1, space="PSUM") as psR,
            tc.tile_pool(name="psM", bufs=2, space="PSUM") as psM,
        ):
            # dummy activation: pulls the exp table load into the DMA fill
            dummy = constp.tile([1, 4], f32, name="dummy")
            nc.gpsimd.memset(dummy[:], 0.0)
            nc.scalar.activation(
                dummy[:, 2:4], dummy[:, 0:2], mybir.ActivationFunctionType.Exp
            )

            gt_sb = constp.tile([E1, HPC * E1], f32r)
            wv_sb = constp.tile([E1, HPC * E1], bf16)
            nc.gpsimd.dma_start(gt_sb[:], gt_d[:])
            nc.gpsimd.dma_start(wv_sb[:], wv_d[:])

            gens = {}  # generation idx -> ring scores tile

            def new_gen(i):
                for k in range(max(gens, default=-1) + 1, i + 1):
                    gens[k] = psR.tile([128, GEN], f32, tag="sc", bufs=RING,
                                       name=f"sc_{k}")
                return gens[i]

            # PE warmup: a 2-col matmul at t~0 so the fill-phase projection
            # matmuls run at ramped pstate instead of cold
            nc.tensor.matmul(
                new_gen(0)[:2, 0:2], dummy[:, 0:2], dummy[:, 2:4],
                start=True, stop=True,
            )

            units = [(b, j) for b in range(B) for j in range(HPC)]

            xts = {}
            xbs = {}

            def fetch_xt(b, j):
                if (b, j) not in xts:
                    for jj in range(HPC):
                        xts[(b, jj)] = sbp.tile(
                            [E1, S], f32r, tag="xt", bufs=4,
                            name=f"xt_{b}_{jj}",
                        )
                        xbs[(b, jj)] = sbp.tile(
                            [E1, S], bf16, tag="xb", bufs=4,
                            name=f"xb_{b}_{jj}",
                        )
                    for jj in range(HPC):  # j-major: head 0 complete first
                        # finer first transfer for unit 0: the first y chunk
                        # and scores piece only need 256 columns
                        spans = [(0, 512), (512, 512)]
                        for s0, w in spans:
                            nc.sync.dma_start(
                                xts[(b, jj)][:, s0:s0 + w],
                                xTe_d[b, jj, :, s0:s0 + w],
                            )
                    for jj in range(HPC):
                        nc.sync.dma_start(xbs[(b, jj)][:], xb_d[b, jj])
                return xts[(b, j)]

            def y_chunks(b, j, fine=False):
                """Scores projection y = G^T.T @ x as filler chunks."""
                xt = fetch_xt(b, j)
                yT = sbp.tile([E1, S], f32r, tag="yT", bufs=3, name=f"y_{b}_{j}")
                chunks = []
                spans = ([(0, 256), (256, 256), (512, 512)] if fine
                         else [(0, 512), (512, 512)])
                for s0, w in spans:
                    def chunk(s0=s0, w=w):
                        y_ps = psM.tile(
                            [128, 512], f32, tag="m", bufs=2, name="y_ps"
                        )
                        nc.tensor.matmul(
                            y_ps[:E1, :w],
                            gt_sb[:, j * E1:(j + 1) * E1],
                            xt[:, s0:s0 + w],
                            start=True, stop=True,
                        )
                        nc.vector.tensor_copy(yT[:, s0:s0 + w], y_ps[:E1, :w])
                    chunks.append(chunk)
                return yT, chunks

            def v_chunks(b, j):
                """v projection as 2 filler chunks (4 bf16 MMs + copy each).

                bf16 x bf16 matmuls run 1 cyc/row even at 66-col outputs
                (f32r would pay 4x below 256 cols), fed by the host-packed
                bf16 copy of the input slice.
                """
                fetch_xt(b, j)
                xb = xbs[(b, j)]
                v_sb = sbp.tile(
                    [128, NT * E1], bf16, tag="v", bufs=4, name=f"v_{b}_{j}"
                )
                chunks = []
                for half in range(2):
                    def chunk(half=half):
                        v_ps = psM.tile(
                            [128, 512], f32, tag="m", bufs=2, name="v_ps"
                        )
                        for q in range(4):
                            tb = half * 4 + q
                            nc.tensor.matmul(
                                v_ps[:, q * E1:(q + 1) * E1],
                                xb[:, tb * 128:(tb + 1) * 128],
                                wv_sb[:, j * E1:(j + 1) * E1],
                                start=True, stop=True,
                            )
                        nc.vector.tensor_copy(
                            v_sb[:, half * 4 * E1:(half + 1) * 4 * E1],
                            v_ps[:, :4 * E1],
                        )
                    chunks.append(chunk)
                return v_sb, chunks

            chunk_exp = {}  # chunk idx -> expT tile

            def emit_exp(c):
                """exp of completed chunk c: ACT Exp, or DVE Schraudolph."""
                csz = CHUNKS[c]
                cp = int(CSTART[c]) % GEN
                src = gens[int(CSTART[c]) // GEN]
                expT = expp.tile(
                    [128, GEN], bf16, tag="expT", bufs=16, name="expT"
                )
                if c in DVE_CHUNKS:
                    nc.vector.tensor_scalar(
                        expT[:, :csz].bitcast(i16),
                        src[:, cp:cp + csz],
                        float(A_EXP),
                        float(B_EXP),
                        mybir.AluOpType.mult,
                        mybir.AluOpType.add,
                    )
                else:
                    nc.scalar.activation(
                        expT[:, :csz], src[:, cp:cp + csz],
                        mybir.ActivationFunctionType.Exp,
                    )
                chunk_exp[c] = expT

            o_sbs = {}

            def get_o(u):
                if u not in o_sbs:
                    b, j = units[u]
                    o_sbs[u] = sbp.tile([128, 512], f32, tag="o", bufs=2,
                                        name=f"o_{b}_{j}")
                return o_sbs[u]

            def pv_mms(u, sh, sbl, out2, v_sb, tbs, first):
                for tb in tbs:
                    c, off = _chunk_of(_col(u, sh, tb, sbl))
                    nc.tensor.matmul(
                        out2,
                        chunk_exp[c][:, off:off + 128],
                        v_sb[:, tb * E1:(tb + 1) * E1],
                        start=(tb == tbs[0] and first),
                        stop=(tb == NT - 1),
                    )

            def scale_sb(u, sh, sbl, out2_64, invz_col, eng="dve"):
                o_sl = get_o(u)[:, (sh * 4 + sbl) * DH:(sh * 4 + sbl + 1) * DH]
                if eng == "act":
                    nc.scalar.activation(
                        o_sl, out2_64,
                        mybir.ActivationFunctionType.Copy,
                        scale=invz_col,
                    )
                else:
                    nc.vector.tensor_scalar_mul(o_sl, out2_64, invz_col)

            def pv_burst(hu, v_sb):
                """PV + normalize + store for half-unit hu (4 s-blocks).

                All four out2 regions live in ONE 1-bank psum tile (the
                accumulation groups run sequentially), so a single strided
                reciprocal covers the four Z columns."""
                u, sh = hu // 2, hu % 2
                shared = {}
                chunks = []
                for sbl in range(4):
                    def chunk(sbl=sbl):
                        if "out2" not in shared:
                            shared["out2"] = psM.tile(
                                [128, 512], f32, tag="m", bufs=2,
                                name=f"pv_{hu}",
                            )
                        out2 = shared["out2"]
                        pv_mms(u, sh, sbl, out2[:, sbl * E1:sbl * E1 + E1],
                               v_sb, list(range(NT)), True)
                    chunks.append(chunk)

                def norms():
                    out2 = shared["out2"]
                    invz = sbp.tile([128, 4], f32, tag="invz", bufs=8,
                                    name="invz")
                    nc.vector.reciprocal(
                        invz[:], out2[:, DH:3 * E1 + DH + 1:E1]
                    )
                    # one 256-col multiply for all 4 s-blocks: the strided
                    # out2 view picks the 64 value cols of each block and
                    # invz broadcasts along the inner dim.
                    o = get_o(u)[:, sh * 256:(sh + 1) * 256]
                    nc.vector.tensor_mul(
                        o.rearrange("p (b e) -> p b e", e=DH),
                        out2[:, :4 * E1].rearrange(
                            "p (b e) -> p b e", e=E1)[:, :, :DH],
                        invz[:].rearrange("p (b e) -> p b e", e=1)
                        .broadcast_to([128, 4, DH]),
                    )
                chunks.append(norms)

                def store():
                    nc.sync.dma_start(
                        out_d[units[u][0], units[u][1]][:, sh * 256:(sh + 1) * 256],
                        get_o(u)[:, sh * 256:(sh + 1) * 256],
                    )
                chunks.append(store)
                return chunks

            # --- last half-unit (hu = 15): staged so almost nothing trails
            # the final exp.  sbl 0,1 park open accumulation groups in the
            # two psM banks (after chunk LAST-3's exp); sbl 2,3 in the two
            # banks of the retired gen tile that chunk LAST-2 read.  After
            # the final exp: 2 matmuls per s-block, 4 normalizes
            # (alternating DVE / ACT), 2 narrow stores.
            last_state = {}

            def last_wave(ceil_c, allowed=(0, 1, 2, 3)):
                u, sh = NU - 1, 1
                v_sb = unit_io[u][1]
                st = last_state
                if "done" not in st:
                    st["done"] = [0] * 4
                    st["out2"] = {}
                    st["started"] = [False] * 4
                chunks = []
                for sbl in allowed:
                    tbs = [tb for tb in range(NT)
                           if st["done"][sbl] <= tb
                           and _chunk_of(_col(u, sh, tb, sbl))[0] <= ceil_c]
                    if not tbs:
                        continue
                    if sbl not in st["out2"]:
                        if sbl < 2:
                            t = psM.tile([128, 512], f32, tag="m", bufs=2,
                                         name=f"lpv_{sbl}")
                            st["out2"][sbl] = t[:, :E1]
                        else:
                            # the tile chunk LAST-2 read is retired once that
                            # exp ran (the final chunk lives in a different
                            # ring slot) -- park sbl 2,3 in its two banks.
                            gi = int(CSTART[LAST - 2]) // GEN
                            off = (sbl - 2) * 512
                            st["out2"][sbl] = new_gen(gi)[:, off:off + E1]
                    out2 = st["out2"][sbl]
                    first = not st["started"][sbl]
                    st["started"][sbl] = True
                    st["done"][sbl] = tbs[-1] + 1

                    def ch(sbl=sbl, tbs=tbs, out2=out2, first=first):
                        pv_mms(u, sh, sbl, out2, v_sb, tbs, first)
                    chunks.append(ch)
                return chunks

            def last_fin():
                u, sh = NU - 1, 1
                b, j = units[u]
                chunks = list(last_wave(LAST))

                def norms_a():
                    st = last_state
                    invz = sbp.tile([128, 2], f32, tag="invz", bufs=8,
                                    name="invz")
                    nc.vector.reciprocal(invz[:, 0:1], st["out2"][0][:, DH:DH + 1])
                    nc.vector.reciprocal(invz[:, 1:2], st["out2"][1][:, DH:DH + 1])
                    scale_sb(u, sh, 0, st["out2"][0][:, :DH], invz[:, 0:1], "dve")
                    scale_sb(u, sh, 1, st["out2"][1][:, :DH], invz[:, 1:2], "act")
                chunks.append(norms_a)

                def norms_b():
                    st = last_state
                    invz = sbp.tile([128, 2], f32, tag="invz", bufs=8,
                                    name="invz")
                    # sbl 2,3 live in the same ring tile: one strided recip
                    g61 = new_gen(int(CSTART[LAST - 2]) // GEN)
                    nc.vector.reciprocal(invz[:], g61[:, DH:DH + 513:512])
                    scale_sb(u, sh, 2, st["out2"][2][:, :DH], invz[:, 0:1], "dve")
                    scale_sb(u, sh, 3, st["out2"][3][:, :DH], invz[:, 1:2], "act")
                chunks.append(norms_b)

                def store_a():
                    nc.sync.dma_start(
                        out_d[b, j][:, 256:384], get_o(u)[:, 256:384]
                    )
                chunks.append(store_a)

                def store_b():
                    nc.sync.dma_start(
                        out_d[b, j][:, 384:512], get_o(u)[:, 384:512]
                    )
                chunks.append(store_b)
                return chunks

            # Software pipeline: scores pieces stream through the psum ring
            # in 512-col steps (s-major within each unit); exp fires per
            # chunk on ACT or DVE; projection chunks of the next unit and
            # PV/normalize/store chunks of completed half-units interleave
            # as fillers.
            from collections import deque

            fillers = deque()
            unit_io = {}

            def unit_inputs(u):
                b, j = units[u]
                yT, ychunks = y_chunks(b, j)
                for c in ychunks:
                    fillers.append(c)
                v_sb, vchunks = v_chunks(b, j)
                for c in vchunks:
                    fillers.append(c)
                unit_io[u] = (yT, v_sb)

            unit_inputs(0)
            u0_chunks = list(fillers)
            fillers.clear()
            for p, (g, w) in enumerate(PIECES):  # scores pieces, s-major
                u = g // 8192
                sh = (g % 8192) // 4096
                tb = (g % 4096) // 512
                so = sh * 512 + (g % 512)  # s-offset within the unit's half
                if p < len(u0_chunks):
                    u0_chunks[p]()  # y chunks before their pieces, then v
                if g % 8192 == 2048 and u + 1 < NU:
                    unit_inputs(u + 1)
                yT, _ = unit_io[u]
                if g % GEN == 0 and g // GEN not in gens:
                    new_gen(g // GEN)
                rp = g % GEN
                nc.tensor.matmul(
                    gens[g // GEN][:, rp:rp + w],
                    fetch_xt(*units[u])[:, tb * 128:(tb + 1) * 128],
                    yT[:, so:so + w],
                    start=True, stop=True,
                )
                c, _ = _chunk_of(g)
                if g + w == int(CSTART[c + 1]):  # chunk complete -> exp
                    emit_exp(c)
                    if c == LAST - 3:
                        # sbl 2,3 park in the tile chunk LAST-2 still reads
                        for ch in last_wave(c, allowed=(0, 1)):
                            fillers.append(ch)
                    elif c in (LAST - 2, LAST - 1):
                        for ch in last_wave(c):
                            fillers.append(ch)
                    elif c == LAST:
                        for ch in last_fin():
                            fillers.append(ch)
                    for hu in range(NHU - 1):
                        if C_END_H[hu] == c:
                            for ch in pv_burst(hu, unit_io[hu // 2][1]):
                                fillers.append(ch)
                for _ in range(2):
                    if fillers:
                        fillers.popleft()()
            while fillers:
                fillers.popleft()()
    if split:
        _split_sync_waits(nc)
    return nc


def _prep_inputs(sequences, Wq, Wk, Wv, bq, bk, bv):
    """Host-side packing: per-core input maps."""
    import ml_dtypes

    sequences = np.ascontiguousarray(np.asarray(sequences, dtype=np.float32))
    Wq = np.asarray(Wq, np.float32)
    Wk = np.asarray(Wk, np.float32)
    Wv = np.asarray(Wv, np.float32)
    bq = np.asarray(bq, np.float32)
    bk = np.asarray(bk, np.float32)
    bv = np.asarray(bv, np.float32)

    # [B, S, H, DH] -> [H, B, DH, S] transposed slices
    xT = np.ascontiguousarray(
        sequences.reshape(B, S, H, DH).transpose(2, 0, 3, 1)
    )  # [H, B, DH, S]

    in_maps = []
    for c in range(NCORES):
        heads = [HPC * c + j for j in range(HPC)]
        xTe = np.zeros((B, HPC, E1, S), np.float32)
        xTe[:, :, DH, :] = 1.0
        for j, h in enumerate(heads):
            xTe[:, j, :DH, :] = xT[h]
        gt = np.zeros((E1, HPC, E1), np.float32)
        wv = np.zeros((E1, HPC, E1), np.float32)
        for j, h in enumerate(heads):
            wq = np.zeros((E1, DH), np.float32)  # x~ -> q, scale folded
            wq[:DH] = Wq[h].T * SCALE
            wq[DH] = bq[h] * SCALE
            wk = np.zeros((E1, DH), np.float32)  # x~ -> k
            wk[:DH] = Wk[h].T
            wk[DH] = bk[h]
            # scores = k.q = x~^T (Wk~ Wq~^T) x~; lhsT of the y-projection
            # is the transpose: G^T = Wq~ @ Wk~^T
            gt[:, j, :] = wq @ wk.T
            wv[:DH, j, :DH] = Wv[h].T
            wv[DH, j, :DH] = bv[h]
            wv[DH, j, DH] = 1.0  # ones column -> Z column of out2
        in_maps.append({
            "xTe": xTe,
            "xbTe": xTe.astype(ml_dtypes.bfloat16),
            "GT": gt.reshape(E1, HPC * E1),
            "WvTe2": wv.reshape(E1, HPC * E1).astype(ml_dtypes.bfloat16),
        })
    return in_maps


def get_nc():
    if "nc" not in _CACHE:
        _CACHE["nc"] = _build_bass()
    return _CACHE["nc"]


def kernel(sequences, Wq, Wk, Wv, bq, bk, bv):
    from concourse.bass_utils import run_bass_kernel_spmd

    nc = get_nc()
    in_maps = _prep_inputs(sequences, Wq, Wk, Wv, bq, bk, bv)
    res = run_bass_kernel_spmd(nc, in_maps, list(range(NCORES)))
    full = np.empty((B, S, D), np.float32)
    for c in range(NCORES):
        # out[b, j, p, blk*64+e] -> full[b, blk*128+p, (2c+j)*64+e]
        arr = res.results[c]["out"].reshape(B, HPC, 128, NT, DH)
        full[:, :, c * HPC * DH:(c + 1) * HPC * DH] = (
            arr.transpose(0, 3, 2, 1, 4).reshape(B, S, HPC * DH)
        )
    return full


# revision 35
# speedup vs baseline: 1.0088x; 1.0079x over previous
"""Multi-head attention (B=4, S=1024, D=1024, H=16, DH=64) on 8 trn2 cores.

Tensor-parallel over heads: core c owns heads {2c, 2c+1}; each core runs
8 independent attention units (4 batches x 2 heads).  Per-head projections
only read a 64-channel slice of the input, so each core receives just its
2x64-channel slice, pre-transposed to [d, s] with a ones-row appended
(E1 = 66: row 64 is the ones row, 65 zero pad).

Math per unit (b, h).  The Wk^T.Wq product is folded on the host
(G^T = Wq~ @ Wk~^T, biases/scale included via the ones-row), so only one
projection feeds the scores:
  y[c,s]    = G^T.T @ xTe           (one 66x66 "projection" replaces q,k)
  scT[t,s]  = xTe.T @ y             (= q.k scores, transposed: t on parts)
  v[t,e']   = xbT.T @ WvTe2         (bf16 inputs: 1 cyc/row; col 64 == 1
                                     -> Z column of out2)
  expT      = exp(scT) -> bf16      (no max-subtraction: |scores| <= ~10)
  out2[s,e']= sum_t expT[t,s] v[t,e']   (transposed PV: s on partitions,
                                     col 64 = Z[s]; per s-block 8
                                     accumulating 66-row bf16 matmuls)
  out[s,e]  = out2[s,e] / Z[s]      (batched 4-way reciprocal + per-sb
                                     tensor_scalar multiply)

Scores stream through a ring of THREE 2-bank PSUM generation tiles
([128, 1024] f32) in 512-col pieces.  exp is split
across TWO engines: ACT chunks use the native Exp; every 4th chunk
(DVE_CHUNKS, 25% of columns) runs on the Vector engine via the
Schraudolph exponent-bit trick -- one tensor_scalar
i16 = round(x * 128/ln2 + B) written through a bf16 bitcast gives
2^(x*log2e) ~ exp(x) with ~3% max element error; since only a quarter
of the keys in each softmax use the approximation the end-to-end
output error stays ~9e-3, under the 2e-2 gate.  The ring depth of 3
is what lets an ACT exp and a DVE exp overlap: pieces of gen g only
wait the exp of gen g-2's chunk.  Engine busy lands near ACT ~50us /
PE ~47us / DVE ~44us instead of the ACT-bound 64us of the
single-engine version (73.3us -> 62.8us end to end).

Pieces are ordered s-major within each unit (s-half, then t-block), so
PV + normalize + store fire per HALF-unit (4 s-blocks, one shared
1-bank psum tile with 4 sequential accumulation groups) ~3 chunks
after the half's scores finish; one strided 4-Z-column reciprocal and
one broadcast tensor_mul normalize 256 output columns per burst.  The
V projection runs on a host-supplied bf16 copy of the input so its
66-col matmuls go 1 cyc/row (f32r pays 4x below 256 cols).  The last
half-unit is staged: sbl 0,1 park open accumulation groups in the two
psM banks (after chunk LAST-3's exp), sbl 2,3 in the retired ring
tile chunk LAST-2 read, so after the final exp only 8 matmuls, 4
normalizes (split ACT/DVE) and 2 narrow stores remain.  A dummy
2-element exp at kernel start pulls the 1.3us activation-table load
into the DMA fill window.
"""

import numpy as np

D = 1024
H = 16
DH = 64
B = 4
S = 1024
NCORES = 8
HPC = H // NCORES  # heads per core = 2
E1 = DH + 2  # 66: ones-row at 64, zero pad at 65
NT = S // 128  # 8 t blocks
NU = B * HPC  # 8 units per core
NHU = NU * 2  # 16 half-units (unit x s-half)
SCALE = 1.0 / np.sqrt(DH)

GEN = 1024  # scores generation: one 2-bank psum tile, in columns
RING = 3  # generations in flight
CHUNKS = [512, 512] + [1024] * 63
N_CH = len(CHUNKS)
LAST = N_CH - 1
assert sum(CHUNKS) == NU * NT * S  # 65536 scores columns per core
CSTART = np.concatenate([[0], np.cumsum(CHUNKS)])
assert all(
    CSTART[i] // GEN == (CSTART[i] + CHUNKS[i] - 1) // GEN
    for i in range(N_CH)
)
# scores pieces: uniform 512 columns (with the PE warmup, finer fill
# pieces no longer pay for their extra instruction overhead)
PIECES = [(512 * k, 512) for k in range(0, 128)]

# chunks whose exp runs on the Vector engine (Schraudolph) instead of ACT:
# every 4th slot offset from the half-unit completion chunks, two extra
# mid-stream slots (32, 48) that rebalance ACT under the warm-PE fill, and
# the FINAL chunk so ACT retires its stream early while the drain overlaps
# the final DVE exp.
DVE_CHUNKS = frozenset(range(2, 59, 4)) | {32, 48, LAST}
A_EXP = 128.0 / np.log(2.0)
B_EXP = 16250.5  # 127*128 - c_opt(~5.5), tuned for hw round-to-nearest

_CACHE = {}


def _chunk_of(g):
    """Chunk index and in-chunk offset for global scores column g."""
    c = int(np.searchsorted(CSTART, g, side="right")) - 1
    return c, g - int(CSTART[c])


def _col(u, sh, tb, sb4=0):
    """Global scores column of unit u, s-half sh, t-block tb, s-subblock."""
    return u * 8192 + sh * 4096 + tb * 512 + sb4 * 128


# chunk whose exp completes each half-unit's scores
C_END_H = [_chunk_of(4096 * (hu + 1) - 1)[0] for hu in range(NHU)]


def _split_sync_waits(nc, limit=1):
    """Walrus in this toolchain rejects instructions carrying more than one
    sync-wait; peel extra waits onto wait-only EventSemaphore ops inserted
    just before, on the same engine queue (engine streams are in-order)."""
    import concourse.mybir as mybir

    n = 0
    for bb in nc.main_func.blocks:
        out = []
        for ins in bb.instructions:
            si = ins.sync_info
            if si is not None and len(si.on_wait) > limit:
                waits = list(si.on_wait)
                for w in waits[:-limit]:
                    ev = mybir.InstEventSemaphore(
                        name=f"WSPLIT-{n}", ins=[], outs=[]
                    )
                    n += 1
                    ev.engine = ins.engine
                    ev.sync_info = mybir.SyncInfo(on_wait=[w], on_update=[])
                    out.append(ev)
                ins.sync_info = mybir.SyncInfo(
                    on_wait=waits[-limit:], on_update=list(si.on_update)
                )
            out.append(ins)
        bb.instructions = out
    return n


def _build_bass(split=True):
    import concourse.bass as bass
    import concourse.mybir as mybir
    import concourse.tile as tile

    f32 = mybir.dt.float32
    f32r = mybir.dt.float32r
    bf16 = mybir.dt.bfloat16
    i16 = mybir.dt.int16
    nc = bass.Bass()

    xTe_d = nc.declare_dram_parameter("xTe", [B, HPC, E1, S], f32r, isOutput=False)
    xb_d = nc.declare_dram_parameter("xbTe", [B, HPC, E1, S], bf16, isOutput=False)
    gt_d = nc.declare_dram_parameter("GT", [E1, HPC * E1], f32r, isOutput=False)
    wv_d = nc.declare_dram_parameter("WvTe2", [E1, HPC * E1], bf16, isOutput=False)
    # out[b, j, p, blk*64 + e] == attention(b, s=blk*128+p, head j)[e]
    out_d = nc.declare_dram_parameter("out", [B, HPC, 128, 512], f32, isOutput=True)

    with tile.TileContext(nc) as tc:
        with (
            tc.tile_pool(name="const", bufs=1) as constp,
            tc.tile_pool(name="sb", bufs=2) as sbp,
            tc.tile_pool(name="expp", bufs=2) as expp,
            tc.tile_pool(name="psR", bufs=1, space="PSUM") as psR,
            tc.tile_pool(name="psM", bufs=2, space="PSUM") as psM,
        ):
            # dummy activation: pulls the exp table load into the DMA fill
            dummy = constp.tile([1, 4], f32, name="dummy")
            nc.gpsimd.memset(dummy[:], 0.0)
            nc.scalar.activation(
                dummy[:, 2:4], dummy[:, 0:2], mybir.ActivationFunctionType.Exp
            )

            gt_sb = constp.tile([E1, HPC * E1], f32r)
            wv_sb = constp.tile([E1, HPC * E1], bf16)
            nc.gpsimd.dma_start(gt_sb[:], gt_d[:])
            nc.gpsimd.dma_start(wv_sb[:], wv_d[:])

            gens = {}  # generation idx -> ring scores tile

            def new_gen(i):
                for k in range(max(gens, default=-1) + 1, i + 1):
                    gens[k] = psR.tile([128, GEN], f32, tag="sc", bufs=RING,
                                       name=f"sc_{k}")
                return gens[i]

            # PE warmup: a 2-col matmul at t~0 so the fill-phase projection
            # matmuls run at ramped pstate instead of cold
            nc.tensor.matmul(
                new_gen(0)[:2, 0:2], dummy[:, 0:2], dummy[:, 2:4],
                start=True, stop=True,
            )

            units = [(b, j) for b in range(B) for j in range(HPC)]

            xts = {}
            xbs = {}

            def fetch_xt(b, j):
                if (b, j) not in xts:
                    for jj in range(HPC):
                        xts[(b, jj)] = sbp.tile(
                            [E1, S], f32r, tag="xt", bufs=4,
                            name=f"xt_{b}_{jj}",
                        )
                        xbs[(b, jj)] = sbp.tile(
                            [E1, S], bf16, tag="xb", bufs=4,
                            name=f"xb_{b}_{jj}",
                        )
                    for jj in range(HPC):  # j-major: head 0 complete first
                        # finer first transfer for unit 0: the first y chunk
                        # and scores piece only need 256 columns
                        spans = [(0, 512), (512, 512)]
                        for s0, w in spans:
                            nc.sync.dma_start(
                                xts[(b, jj)][:, s0:s0 + w],
                                xTe_d[b, jj, :, s0:s0 + w],
                            )
                    for jj in range(HPC):
                        nc.sync.dma_start(xbs[(b, jj)][:], xb_d[b, jj])
                return xts[(b, j)]

            def y_chunks(b, j, fine=False):
                """Scores projection y = G^T.T @ x as filler chunks."""
                xt = fetch_xt(b, j)
                yT = sbp.tile([E1, S], f32r, tag="yT", bufs=3, name=f"y_{b}_{j}")
                chunks = []
                spans = ([(0, 256), (256, 256), (512, 512)] if fine
                         else [(0, 512), (512, 512)])
                for s0, w in spans:
                    def chunk(s0=s0, w=w):
                        y_ps = psM.tile(
                            [128, 512], f32, tag="m", bufs=2, name="y_ps"
                        )
                        nc.tensor.matmul(
                            y_ps[:E1, :w],
                            gt_sb[:, j * E1:(j + 1) * E1],
                            xt[:, s0:s0 + w],
                            start=True, stop=True,
                        )
                        nc.vector.tensor_copy(yT[:, s0:s0 + w], y_ps[:E1, :w])
                    chunks.append(chunk)
                return yT, chunks

            def v_chunks(b, j):
                """v projection as 2 filler chunks (4 bf16 MMs + copy each).

                bf16 x bf16 matmuls run 1 cyc/row even at 66-col outputs
                (f32r would pay 4x below 256 cols), fed by the host-packed
                bf16 copy of the input slice.
                """
                fetch_xt(b, j)
                xb = xbs[(b, j)]
                v_sb = sbp.tile(
                    [128, NT * E1], bf16, tag="v", bufs=4, name=f"v_{b}_{j}"
                )
                chunks = []
                for half in range(2):
                    def chunk(half=half):
                        v_ps = psM.tile(
                            [128, 512], f32, tag="m", bufs=2, name="v_ps"
                        )
                        for q in range(4):
                            tb = half * 4 + q
                            nc.tensor.matmul(
                                v_ps[:, q * E1:(q + 1) * E1],
                                xb[:, tb * 128:(tb + 1) * 128],
                                wv_sb[:, j * E1:(j + 1) * E1],
                                start=True, stop=True,
                            )
                        nc.vector.tensor_copy(
                            v_sb[:, half * 4 * E1:(half + 1) * 4 * E1],
                            v_ps[:, :4 * E1],
                        )
                    chunks.append(chunk)
                return v_sb, chunks

            chunk_exp = {}  # chunk idx -> expT tile

            def emit_exp(c):
                """exp of completed chunk c: ACT Exp, or DVE Schraudolph."""
                csz = CHUNKS[c]
                cp = int(CSTART[c]) % GEN
                src = gens[int(CSTART[c]) // GEN]
                expT = expp.tile(
                    [128, GEN], bf16, tag="expT", bufs=16, name="expT"
                )
                if c in DVE_CHUNKS:
                    nc.vector.tensor_scalar(
                        expT[:, :csz].bitcast(i16),
                        src[:, cp:cp + csz],
                        float(A_EXP),
                        float(B_EXP),
                        mybir.AluOpType.mult,
                        mybir.AluOpType.add,
                    )
                else:
                    nc.scalar.activation(
                        expT[:, :csz], src[:, cp:cp + csz],
                        mybir.ActivationFunctionType.Exp,
                    )
                chunk_exp[c] = expT

            o_sbs = {}

            def get_o(u):
                if u not in o_sbs:
                    b, j = units[u]
                    o_sbs[u] = sbp.tile([128, 512], f32, tag="o", bufs=2,
                                        name=f"o_{b}_{j}")
                return o_sbs[u]

            def pv_mms(u, sh, sbl, out2, v_sb, tbs, first):
                for tb in tbs:
                    c, off = _chunk_of(_col(u, sh, tb, sbl))
                    nc.tensor.matmul(
                        out2,
                        chunk_exp[c][:, off:off + 128],
                        v_sb[:, tb * E1:(tb + 1) * E1],
                        start=(tb == tbs[0] and first),
                        stop=(tb == NT - 1),
                    )

            def scale_sb(u, sh, sbl, out2_64, invz_col, eng="dve"):
                o_sl = get_o(u)[:, (sh * 4 + sbl) * DH:(sh * 4 + sbl + 1) * DH]
                if eng == "act":
                    nc.scalar.activation(
                        o_sl, out2_64,
                        mybir.ActivationFunctionType.Copy,
                        scale=invz_col,
                    )
                else:
                    nc.vector.tensor_scalar_mul(o_sl, out2_64, invz_col)

            def pv_burst(hu, v_sb):
                """PV + normalize + store for half-unit hu (4 s-blocks).

                All four out2 regions live in ONE 1-bank psum tile (the
                accumulation groups run sequentially), so a single strided
                reciprocal covers the four Z columns."""
                u, sh = hu // 2, hu % 2
                shared = {}
                chunks = []
                for sbl in range(4):
                    def chunk(sbl=sbl):
                        if "out2" not in shared:
                            shared["out2"] = psM.tile(
                                [128, 512], f32, tag="m", bufs=2,
                                name=f"pv_{hu}",
                            )
                        out2 = shared["out2"]
                        pv_mms(u, sh, sbl, out2[:, sbl * E1:sbl * E1 + E1],
                               v_sb, list(range(NT)), True)
                    chunks.append(chunk)

                def norms():
                    out2 = shared["out2"]
                    invz = sbp.tile([128, 4], f32, tag="invz", bufs=8,
                                    name="invz")
                    nc.vector.reciprocal(
                        invz[:], out2[:, DH:3 * E1 + DH + 1:E1]
                    )
                    # one 256-col multiply for all 4 s-blocks: the strided
                    # out2 view picks the 64 value cols of each block and
                    # invz broadcasts along the inner dim.
                    o = get_o(u)[:, sh * 256:(sh + 1) * 256]
                    nc.vector.tensor_mul(
                        o.rearrange("p (b e) -> p b e", e=DH),
                        out2[:, :4 * E1].rearrange(
                            "p (b e) -> p b e", e=E1)[:, :, :DH],
                        invz[:].rearrange("p (b e) -> p b e", e=1)
                        .broadcast_to([128, 4, DH]),
                    )
                chunks.append(norms)

                def store():
                    nc.sync.dma_start(
                        out_d[units[u][0], units[u][1]][:, sh * 256:(sh + 1) * 256],
                        get_o(u)[:, sh * 256:(sh + 1) * 256],
                    )
                chunks.append(store)
                return chunks

            # --- last half-unit (hu = 15): staged so almost nothing trails
            # the final exp.  sbl 0,1 park open accumulation groups in the
            # two psM banks (after chunk LAST-3's exp); sbl 2,3 in the two
            # banks of the retired gen tile that chunk LAST-2 read.  After
            # the final exp: 2 matmuls per s-block, 4 normalizes
            # (alternating DVE / ACT), 2 narrow stores.
            last_state = {}

            def last_wave(ceil_c, allowed=(0, 1, 2, 3)):
                u, sh = NU - 1, 1
                v_sb = unit_io[u][1]
                st = last_state
                if "done" not in st:
                    st["done"] = [0] * 4
                    st["out2"] = {}
                    st["started"] = [False] * 4
                chunks = []
                for sbl in allowed:
                    tbs = [tb for tb in range(NT)
                           if st["done"][sbl] <= tb
                           and _chunk_of(_col(u, sh, tb, sbl))[0] <= ceil_c]
                    if not tbs:
                        continue
                    if sbl not in st["out2"]:
                        if sbl < 2:
                            t = psM.tile([128, 512], f32, tag="m", bufs=2,
                                         name=f"lpv_{sbl}")
                            st["out2"][sbl] = t[:, :E1]
                        else:
                            # the tile chunk LAST-2 read is retired once that
                            # exp ran (the final chunk lives in a different
                            # ring slot) -- park sbl 2,3 in its two banks.
                            gi = int(CSTART[LAST - 2]) // GEN
                            off = (sbl - 2) * 512
                            st["out2"][sbl] = new_gen(gi)[:, off:off + E1]
                    out2 = st["out2"][sbl]
                    first = not st["started"][sbl]
                    st["started"][sbl] = True
                    st["done"][sbl] = tbs[-1] + 1

                    def ch(sbl=sbl, tbs=tbs, out2=out2, first=first):
                        pv_mms(u, sh, sbl, out2, v_sb, tbs, first)
                    chunks.append(ch)
                return chunks

            def last_fin():
                u, sh = NU - 1, 1
                b, j = units[u]
                chunks = list(last_wave(LAST))

                def norms_a():
                    st = last_state
                    invz = sbp.tile([128, 2], f32, tag="invz", bufs=8,
                                    name="invz")
                    nc.vector.reciprocal(invz[:, 0:1], st["out2"][0][:, DH:DH + 1])
                    nc.vector.reciprocal(invz[:, 1:2], st["out2"][1][:, DH:DH + 1])
                    scale_sb(u, sh, 0, st["out2"][0][:, :DH], invz[:, 0:1], "dve")
                    scale_sb(u, sh, 1, st["out2"][1][:, :DH], invz[:, 1:2], "act")
                chunks.append(norms_a)

                def norms_b():
                    st = last_state
                    invz = sbp.tile([128, 2], f32, tag="invz", bufs=8,
                                    name="invz")
                    # sbl 2,3 live in the same ring tile: one strided recip
                    g61 = new_gen(int(CSTART[LAST - 2]) // GEN)
                    nc.vector.reciprocal(invz[:], g61[:, DH:DH + 513:512])
                    scale_sb(u, sh, 2, st["out2"][2][:, :DH], invz[:, 0:1], "dve")
                    scale_sb(u, sh, 3, st["out2"][3][:, :DH], invz[:, 1:2], "act")
                chunks.append(norms_b)

                def store_a():
                    nc.sync.dma_start(
                        out_d[b, j][:, 256:384], get_o(u)[:, 256:384]
                    )
                chunks.append(store_a)

                def store_b():
                    nc.sync.dma_start(
                        out_d[b, j][:, 384:512], get_o(u)[:, 384:512]
                    )
                chunks.append(store_b)
                return chunks

            # Software pipeline: scores pieces stream through the psum ring
            # in 512-col steps (s-major within each unit); exp fires per
            # chunk on ACT or DVE; projection chunks of the next unit and
            # PV/normalize/store chunks of completed half-units interleave
            # as fillers.
            from collections import deque

            fillers = deque()
            unit_io = {}

            def unit_inputs(u):
                b, j = units[u]
                yT, ychunks = y_chunks(b, j)
                for c in ychunks:
                    fillers.append(c)
                v_sb, vchunks = v_chunks(b, j)
                for c in vchunks:
                    fillers.append(c)
                unit_io[u] = (yT, v_sb)

            unit_inputs(0)
            u0_chunks = list(fillers)
            fillers.clear()
            for p, (g, w) in enumerate(PIECES):  # scores pieces, s-major
                u = g // 8192
                sh = (g % 8192) // 4096
                tb = (g % 4096) // 512
                so = sh * 512 + (g % 512)  # s-offset within the unit's half
                if p < len(u0_chunks):
                    u0_chunks[p]()  # y chunks before their pieces, then v
                if g % 8192 == 2048 and u + 1 < NU:
                    unit_inputs(u + 1)
                yT, _ = unit_io[u]
                if g % GEN == 0 and g // GEN not in gens:
                    new_gen(g // GEN)
                rp = g % GEN
                nc.tensor.matmul(
                    gens[g // GEN][:, rp:rp + w],
                    fetch_xt(*units[u])[:, tb * 128:(tb + 1) * 128],
                    yT[:, so:so + w],
                    start=True, stop=True,
                )
                c, _ = _chunk_of(g)
                if g + w == int(CSTART[c + 1]):  # chunk complete -> exp
                    emit_exp(c)
                    if c == LAST - 3:
                        # sbl 2,3 park in the tile chunk LAST-2 still reads
                        for ch in last_wave(c, allowed=(0, 1)):
                            fillers.append(ch)
                    elif c in (LAST - 2, LAST - 1):
                        for ch in last_wave(c):
                            fillers.append(ch)
                    elif c == LAST:
                        for ch in last_fin():
                            fillers.append(ch)
                    for hu in range(NHU - 1):
                        if C_END_H[hu] == c:
                            for ch in pv_burst(hu, unit_io[hu // 2][1]):
                                fillers.append(ch)
                for _ in range(2):
                    if fillers:
                        fillers.popleft()()
            while fillers:
                fillers.popleft()()
    if split:
        _split_sync_waits(nc)
    return nc


def _prep_inputs(sequences, Wq, Wk, Wv, bq, bk, bv):
    """Host-side packing: per-core input maps."""
    import ml_dtypes

    sequences = np.ascontiguousarray(np.asarray(sequences, dtype=np.float32))
    Wq = np.asarray(Wq, np.float32)
    Wk = np.asarray(Wk, np.float32)
    Wv = np.asarray(Wv, np.float32)
    bq = np.asarray(bq, np.float32)
    bk = np.asarray(bk, np.float32)
    bv = np.asarray(bv, np.float32)

    # [B, S, H, DH] -> [H, B, DH, S] transposed slices
    xT = np.ascontiguousarray(
        sequences.reshape(B, S, H, DH).transpose(2, 0, 3, 1)
    )  # [H, B, DH, S]

    in_maps = []
    for c in range(NCORES):
        heads = [HPC * c + j for j in range(HPC)]
        xTe = np.zeros((B, HPC, E1, S), np.float32)
        xTe[:, :, DH, :] = 1.0
        for j, h in enumerate(heads):
            xTe[:, j, :DH, :] = xT[h]
        gt = np.zeros((E1, HPC, E1), np.float32)
        wv = np.zeros((E1, HPC, E1), np.float32)
        for j, h in enumerate(heads):
            wq = np.zeros((E1, DH), np.float32)  # x~ -> q, scale folded
            wq[:DH] = Wq[h].T * SCALE
            wq[DH] = bq[h] * SCALE
            wk = np.zeros((E1, DH), np.float32)  # x~ -> k
            wk[:DH] = Wk[h].T
            wk[DH] = bk[h]
            # scores = k.q = x~^T (Wk~ Wq~^T) x~; lhsT of the y-projection
            # is the transpose: G^T = Wq~ @ Wk~^T
            gt[:, j, :] = wq @ wk.T
            wv[:DH, j, :DH] = Wv[h].T
            wv[DH, j, :DH] = bv[h]
            wv[DH, j, DH] = 1.0  # ones column -> Z column of out2
        in_maps.append({
            "xTe": xTe,
            "xbTe": xTe.astype(ml_dtypes.bfloat16),
            "GT": gt.reshape(E1, HPC * E1),
            "WvTe2": wv.reshape(E1, HPC * E1).astype(ml_dtypes.bfloat16),
        })
    return in_maps


def get_nc():
    if "nc" not in _CACHE:
        _CACHE["nc"] = _build_bass()
    return _CACHE["nc"]


def kernel(sequences, Wq, Wk, Wv, bq, bk, bv):
    from concourse.bass_utils import run_bass_kernel_spmd

    nc = get_nc()
    in_maps = _prep_inputs(sequences, Wq, Wk, Wv, bq, bk, bv)
    res = run_bass_kernel_spmd(nc, in_maps, list(range(NCORES)))
    full = np.empty((B, S, D), np.float32)
    for c in range(NCORES):
        # out[b, j, p, blk*64+e] -> full[b, blk*128+p, (2c+j)*64+e]
        arr = res.results[c]["out"].reshape(B, HPC, 128, NT, DH)
        full[:, :, c * HPC * DH:(c + 1) * HPC * DH] = (
            arr.transpose(0, 3, 2, 1, 4).reshape(B, S, HPC * DH)
        )
    return full


# revision 36
# speedup vs baseline: 1.0106x; 1.0018x over previous
"""Multi-head attention (B=4, S=1024, D=1024, H=16, DH=64) on 8 trn2 cores.

Tensor-parallel over heads: core c owns heads {2c, 2c+1}; each core runs
8 independent attention units (4 batches x 2 heads).  Per-head projections
only read a 64-channel slice of the input, so each core receives just its
2x64-channel slice, pre-transposed to [d, s] with a ones-row appended
(E1 = 66: row 64 is the ones row, 65 zero pad).

Math per unit (b, h).  The Wk^T.Wq product is folded on the host
(G^T = Wq~ @ Wk~^T, biases/scale included via the ones-row), so only one
projection feeds the scores:
  y[c,s]    = G^T.T @ xTe           (one 66x66 "projection" replaces q,k)
  scT[t,s]  = xTe.T @ y             (= q.k scores, transposed: t on parts)
  v[t,e']   = xbT.T @ WvTe2         (bf16 inputs: 1 cyc/row; col 64 == 1
                                     -> Z column of out2)
  expT      = exp(scT) -> bf16      (no max-subtraction: |scores| <= ~10)
  out2[s,e']= sum_t expT[t,s] v[t,e']   (transposed PV: s on partitions,
                                     col 64 = Z[s]; per s-block 8
                                     accumulating 66-row bf16 matmuls)
  out[s,e]  = out2[s,e] / Z[s]      (batched 4-way reciprocal + per-sb
                                     tensor_scalar multiply)

Scores stream through a ring of THREE 2-bank PSUM generation tiles
([128, 1024] f32) in 512-col pieces.  exp is split
across TWO engines: ACT chunks use the native Exp; every 4th chunk
(DVE_CHUNKS, 25% of columns) runs on the Vector engine via the
Schraudolph exponent-bit trick -- one tensor_scalar
i16 = round(x * 128/ln2 + B) written through a bf16 bitcast gives
2^(x*log2e) ~ exp(x) with ~3% max element error; since only a quarter
of the keys in each softmax use the approximation the end-to-end
output error stays ~9e-3, under the 2e-2 gate.  The ring depth of 3
is what lets an ACT exp and a DVE exp overlap: pieces of gen g only
wait the exp of gen g-2's chunk.  Engine busy lands near ACT ~50us /
PE ~47us / DVE ~44us instead of the ACT-bound 64us of the
single-engine version (73.3us -> 62.8us end to end).

Pieces are ordered s-major within each unit (s-half, then t-block), so
PV + normalize + store fire per HALF-unit (4 s-blocks, one shared
1-bank psum tile with 4 sequential accumulation groups) ~3 chunks
after the half's scores finish; one strided 4-Z-column reciprocal and
one broadcast tensor_mul normalize 256 output columns per burst.  The
V projection runs on a host-supplied bf16 copy of the input so its
66-col matmuls go 1 cyc/row (f32r pays 4x below 256 cols).  The last
half-unit is staged: sbl 0,1 park open accumulation groups in the two
psM banks (after chunk LAST-3's exp), sbl 2,3 in the retired ring
tile chunk LAST-2 read, so after the final exp only 8 matmuls, 4
normalizes (split ACT/DVE) and 2 narrow stores remain.  A dummy
2-element exp at kernel start pulls the 1.3us activation-table load
into the DMA fill window.
"""

import numpy as np

D = 1024
H = 16
DH = 64
B = 4
S = 1024
NCORES = 8
HPC = H // NCORES  # heads per core = 2
E1 = DH + 2  # 66: ones-row at 64, zero pad at 65
NT = S // 128  # 8 t blocks
NU = B * HPC  # 8 units per core
NHU = NU * 2  # 16 half-units (unit x s-half)
SCALE = 1.0 / np.sqrt(DH)

GEN = 1024  # scores generation: one 2-bank psum tile, in columns
RING = 3  # generations in flight
CHUNKS = [512, 512] + [1024] * 63
N_CH = len(CHUNKS)
LAST = N_CH - 1
assert sum(CHUNKS) == NU * NT * S  # 65536 scores columns per core
CSTART = np.concatenate([[0], np.cumsum(CHUNKS)])
assert all(
    CSTART[i] // GEN == (CSTART[i] + CHUNKS[i] - 1) // GEN
    for i in range(N_CH)
)
# scores pieces: uniform 512 columns (with the PE warmup, finer fill
# pieces no longer pay for their extra instruction overhead)
PIECES = [(512 * k, 512) for k in range(0, 128)]

# chunks whose exp runs on the Vector engine (Schraudolph) instead of ACT:
# every 4th slot offset from the half-unit completion chunks, two extra
# mid-stream slots (16, 32, 48) that rebalance ACT under the warm-PE fill,
# and the FINAL chunk so ACT retires its stream early while the drain
# overlaps the final DVE exp.
DVE_CHUNKS = frozenset(range(2, 59, 4)) | {16, 32, 48, LAST}
A_EXP = 128.0 / np.log(2.0)
B_EXP = 16250.5  # 127*128 - c_opt(~5.5), tuned for hw round-to-nearest

_CACHE = {}


def _chunk_of(g):
    """Chunk index and in-chunk offset for global scores column g."""
    c = int(np.searchsorted(CSTART, g, side="right")) - 1
    return c, g - int(CSTART[c])


def _col(u, sh, tb, sb4=0):
    """Global scores column of unit u, s-half sh, t-block tb, s-subblock."""
    return u * 8192 + sh * 4096 + tb * 512 + sb4 * 128


# chunk whose exp completes each half-unit's scores
C_END_H = [_chunk_of(4096 * (hu + 1) - 1)[0] for hu in range(NHU)]


def _split_sync_waits(nc, limit=1):
    """Walrus in this toolchain rejects instructions carrying more than one
    sync-wait; peel extra waits onto wait-only EventSemaphore ops inserted
    just before, on the same engine queue (engine streams are in-order)."""
    import concourse.mybir as mybir

    n = 0
    for bb in nc.main_func.blocks:
        out = []
        for ins in bb.instructions:
            si = ins.sync_info
            if si is not None and len(si.on_wait) > limit:
                waits = list(si.on_wait)
                for w in waits[:-limit]:
                    ev = mybir.InstEventSemaphore(
                        name=f"WSPLIT-{n}", ins=[], outs=[]
                    )
                    n += 1
                    ev.engine = ins.engine
                    ev.sync_info = mybir.SyncInfo(on_wait=[w], on_update=[])
                    out.append(ev)
                ins.sync_info = mybir.SyncInfo(
                    on_wait=waits[-limit:], on_update=list(si.on_update)
                )
            out.append(ins)
        bb.instructions = out
    return n


def _build_bass(split=True):
    import concourse.bass as bass
    import concourse.mybir as mybir
    import concourse.tile as tile

    f32 = mybir.dt.float32
    f32r = mybir.dt.float32r
    bf16 = mybir.dt.bfloat16
    i16 = mybir.dt.int16
    nc = bass.Bass()

    xTe_d = nc.declare_dram_parameter("xTe", [B, HPC, E1, S], f32r, isOutput=False)
    xb_d = nc.declare_dram_parameter("xbTe", [B, HPC, E1, S], bf16, isOutput=False)
    gt_d = nc.declare_dram_parameter("GT", [E1, HPC * E1], f32r, isOutput=False)
    wv_d = nc.declare_dram_parameter("WvTe2", [E1, HPC * E1], bf16, isOutput=False)
    # out[b, j, p, blk*64 + e] == attention(b, s=blk*128+p, head j)[e]
    out_d = nc.declare_dram_parameter("out", [B, HPC, 128, 512], f32, isOutput=True)

    with tile.TileContext(nc) as tc:
        with (
            tc.tile_pool(name="const", bufs=1) as constp,
            tc.tile_pool(name="sb", bufs=2) as sbp,
            tc.tile_pool(name="expp", bufs=2) as expp,
            tc.tile_pool(name="psR", bufs=1, space="PSUM") as psR,
            tc.tile_pool(name="psM", bufs=2, space="PSUM") as psM,
        ):
            # dummy activation: pulls the exp table load into the DMA fill
            dummy = constp.tile([1, 4], f32, name="dummy")
            nc.gpsimd.memset(dummy[:], 0.0)
            nc.scalar.activation(
                dummy[:, 2:4], dummy[:, 0:2], mybir.ActivationFunctionType.Exp
            )

            gt_sb = constp.tile([E1, HPC * E1], f32r)
            wv_sb = constp.tile([E1, HPC * E1], bf16)
            nc.gpsimd.dma_start(gt_sb[:], gt_d[:])
            nc.gpsimd.dma_start(wv_sb[:], wv_d[:])

            gens = {}  # generation idx -> ring scores tile

            def new_gen(i):
                for k in range(max(gens, default=-1) + 1, i + 1):
                    gens[k] = psR.tile([128, GEN], f32, tag="sc", bufs=RING,
                                       name=f"sc_{k}")
                return gens[i]

            # PE warmup: a 2-col matmul at t~0 so the fill-phase projection
            # matmuls run at ramped pstate instead of cold
            nc.tensor.matmul(
                new_gen(0)[:2, 0:2], dummy[:, 0:2], dummy[:, 2:4],
                start=True, stop=True,
            )

            units = [(b, j) for b in range(B) for j in range(HPC)]

            xts = {}
            xbs = {}

            def fetch_xt(b, j):
                if (b, j) not in xts:
                    for jj in range(HPC):
                        xts[(b, jj)] = sbp.tile(
                            [E1, S], f32r, tag="xt", bufs=4,
                            name=f"xt_{b}_{jj}",
                        )
                        xbs[(b, jj)] = sbp.tile(
                            [E1, S], bf16, tag="xb", bufs=4,
                            name=f"xb_{b}_{jj}",
                        )
                    for jj in range(HPC):  # j-major: head 0 complete first
                        # finer first transfer for unit 0: the first y chunk
                        # and scores piece only need 256 columns
                        spans = [(0, 512), (512, 512)]
                        for s0, w in spans:
                            nc.sync.dma_start(
                                xts[(b, jj)][:, s0:s0 + w],
                                xTe_d[b, jj, :, s0:s0 + w],
                            )
                    for jj in range(HPC):
                        nc.sync.dma_start(xbs[(b, jj)][:], xb_d[b, jj])
                return xts[(b, j)]

            def y_chunks(b, j, fine=False):
                """Scores projection y = G^T.T @ x as filler chunks."""
                xt = fetch_xt(b, j)
                yT = sbp.tile([E1, S], f32r, tag="yT", bufs=3, name=f"y_{b}_{j}")
                chunks = []
                spans = ([(0, 256), (256, 256), (512, 512)] if fine
                         else [(0, 512), (512, 512)])
                for s0, w in spans:
                    def chunk(s0=s0, w=w):
                        y_ps = psM.tile(
                            [128, 512], f32, tag="m", bufs=2, name="y_ps"
                        )
                        nc.tensor.matmul(
                            y_ps[:E1, :w],
                            gt_sb[:, j * E1:(j + 1) * E1],
                            xt[:, s0:s0 + w],
                            start=True, stop=True,
                        )
                        nc.vector.tensor_copy(yT[:, s0:s0 + w], y_ps[:E1, :w])
                    chunks.append(chunk)
                return yT, chunks

            def v_chunks(b, j):
                """v projection as 2 filler chunks (4 bf16 MMs + copy each).

                bf16 x bf16 matmuls run 1 cyc/row even at 66-col outputs
                (f32r would pay 4x below 256 cols), fed by the host-packed
                bf16 copy of the input slice.
                """
                fetch_xt(b, j)
                xb = xbs[(b, j)]
                v_sb = sbp.tile(
                    [128, NT * E1], bf16, tag="v", bufs=4, name=f"v_{b}_{j}"
                )
                chunks = []
                for half in range(2):
                    def chunk(half=half):
                        v_ps = psM.tile(
                            [128, 512], f32, tag="m", bufs=2, name="v_ps"
                        )
                        for q in range(4):
                            tb = half * 4 + q
                            nc.tensor.matmul(
                                v_ps[:, q * E1:(q + 1) * E1],
                                xb[:, tb * 128:(tb + 1) * 128],
                                wv_sb[:, j * E1:(j + 1) * E1],
                                start=True, stop=True,
                            )
                        nc.vector.tensor_copy(
                            v_sb[:, half * 4 * E1:(half + 1) * 4 * E1],
                            v_ps[:, :4 * E1],
                        )
                    chunks.append(chunk)
                return v_sb, chunks

            chunk_exp = {}  # chunk idx -> expT tile

            def emit_exp(c):
                """exp of completed chunk c: ACT Exp, or DVE Schraudolph."""
                csz = CHUNKS[c]
                cp = int(CSTART[c]) % GEN
                src = gens[int(CSTART[c]) // GEN]
                expT = expp.tile(
                    [128, GEN], bf16, tag="expT", bufs=16, name="expT"
                )
                if c in DVE_CHUNKS:
                    nc.vector.tensor_scalar(
                        expT[:, :csz].bitcast(i16),
                        src[:, cp:cp + csz],
                        float(A_EXP),
                        float(B_EXP),
                        mybir.AluOpType.mult,
                        mybir.AluOpType.add,
                    )
                else:
                    nc.scalar.activation(
                        expT[:, :csz], src[:, cp:cp + csz],
                        mybir.ActivationFunctionType.Exp,
                    )
                chunk_exp[c] = expT

            o_sbs = {}

            def get_o(u):
                if u not in o_sbs:
                    b, j = units[u]
                    o_sbs[u] = sbp.tile([128, 512], f32, tag="o", bufs=2,
                                        name=f"o_{b}_{j}")
                return o_sbs[u]

            def pv_mms(u, sh, sbl, out2, v_sb, tbs, first):
                for tb in tbs:
                    c, off = _chunk_of(_col(u, sh, tb, sbl))
                    nc.tensor.matmul(
                        out2,
                        chunk_exp[c][:, off:off + 128],
                        v_sb[:, tb * E1:(tb + 1) * E1],
                        start=(tb == tbs[0] and first),
                        stop=(tb == NT - 1),
                    )

            def scale_sb(u, sh, sbl, out2_64, invz_col, eng="dve"):
                o_sl = get_o(u)[:, (sh * 4 + sbl) * DH:(sh * 4 + sbl + 1) * DH]
                if eng == "act":
                    nc.scalar.activation(
                        o_sl, out2_64,
                        mybir.ActivationFunctionType.Copy,
                        scale=invz_col,
                    )
                else:
                    nc.vector.tensor_scalar_mul(o_sl, out2_64, invz_col)

            def pv_burst(hu, v_sb):
                """PV + normalize + store for half-unit hu (4 s-blocks).

                All four out2 regions live in ONE 1-bank psum tile (the
                accumulation groups run sequentially), so a single strided
                reciprocal covers the four Z columns."""
                u, sh = hu // 2, hu % 2
                shared = {}
                chunks = []
                for sbl in range(4):
                    def chunk(sbl=sbl):
                        if "out2" not in shared:
                            shared["out2"] = psM.tile(
                                [128, 512], f32, tag="m", bufs=2,
                                name=f"pv_{hu}",
                            )
                        out2 = shared["out2"]
                        pv_mms(u, sh, sbl, out2[:, sbl * E1:sbl * E1 + E1],
                               v_sb, list(range(NT)), True)
                    chunks.append(chunk)

                def norms():
                    out2 = shared["out2"]
                    invz = sbp.tile([128, 4], f32, tag="invz", bufs=8,
                                    name="invz")
                    nc.vector.reciprocal(
                        invz[:], out2[:, DH:3 * E1 + DH + 1:E1]
                    )
                    # one 256-col multiply for all 4 s-blocks: the strided
                    # out2 view picks the 64 value cols of each block and
                    # invz broadcasts along the inner dim.
                    o = get_o(u)[:, sh * 256:(sh + 1) * 256]
                    nc.vector.tensor_mul(
                        o.rearrange("p (b e) -> p b e", e=DH),
                        out2[:, :4 * E1].rearrange(
                            "p (b e) -> p b e", e=E1)[:, :, :DH],
                        invz[:].rearrange("p (b e) -> p b e", e=1)
                        .broadcast_to([128, 4, DH]),
                    )
                chunks.append(norms)

                def store():
                    nc.sync.dma_start(
                        out_d[units[u][0], units[u][1]][:, sh * 256:(sh + 1) * 256],
                        get_o(u)[:, sh * 256:(sh + 1) * 256],
                    )
                chunks.append(store)
                return chunks

            # --- last half-unit (hu = 15): staged so almost nothing trails
            # the final exp.  sbl 0,1 park open accumulation groups in the
            # two psM banks (after chunk LAST-3's exp); sbl 2,3 in the two
            # banks of the retired gen tile that chunk LAST-2 read.  After
            # the final exp: 2 matmuls per s-block, 4 normalizes
            # (alternating DVE / ACT), 2 narrow stores.
            last_state = {}

            def last_wave(ceil_c, allowed=(0, 1, 2, 3)):
                u, sh = NU - 1, 1
                v_sb = unit_io[u][1]
                st = last_state
                if "done" not in st:
                    st["done"] = [0] * 4
                    st["out2"] = {}
                    st["started"] = [False] * 4
                chunks = []
                for sbl in allowed:
                    tbs = [tb for tb in range(NT)
                           if st["done"][sbl] <= tb
                           and _chunk_of(_col(u, sh, tb, sbl))[0] <= ceil_c]
                    if not tbs:
                        continue
                    if sbl not in st["out2"]:
                        if sbl < 2:
                            t = psM.tile([128, 512], f32, tag="m", bufs=2,
                                         name=f"lpv_{sbl}")
                            st["out2"][sbl] = t[:, :E1]
                        else:
                            # the tile chunk LAST-2 read is retired once that
                            # exp ran (the final chunk lives in a different
                            # ring slot) -- park sbl 2,3 in its two banks.
                            gi = int(CSTART[LAST - 2]) // GEN
                            off = (sbl - 2) * 512
                            st["out2"][sbl] = new_gen(gi)[:, off:off + E1]
                    out2 = st["out2"][sbl]
                    first = not st["started"][sbl]
                    st["started"][sbl] = True
                    st["done"][sbl] = tbs[-1] + 1

                    def ch(sbl=sbl, tbs=tbs, out2=out2, first=first):
                        pv_mms(u, sh, sbl, out2, v_sb, tbs, first)
                    chunks.append(ch)
                return chunks

            def last_fin():
                u, sh = NU - 1, 1
                b, j = units[u]
                chunks = list(last_wave(LAST))

                def norms_a():
                    st = last_state
                    invz = sbp.tile([128, 2], f32, tag="invz", bufs=8,
                                    name="invz")
                    nc.vector.reciprocal(invz[:, 0:1], st["out2"][0][:, DH:DH + 1])
                    nc.vector.reciprocal(invz[:, 1:2], st["out2"][1][:, DH:DH + 1])
                    scale_sb(u, sh, 0, st["out2"][0][:, :DH], invz[:, 0:1], "dve")
                    scale_sb(u, sh, 1, st["out2"][1][:, :DH], invz[:, 1:2], "act")
                chunks.append(norms_a)

                def norms_b():
                    st = last_state
                    invz = sbp.tile([128, 2], f32, tag="invz", bufs=8,
                                    name="invz")
                    # sbl 2,3 live in the same ring tile: one strided recip
                    g61 = new_gen(int(CSTART[LAST - 2]) // GEN)
                    nc.vector.reciprocal(invz[:], g61[:, DH:DH + 513:512])
                    scale_sb(u, sh, 2, st["out2"][2][:, :DH], invz[:, 0:1], "dve")
                    scale_sb(u, sh, 3, st["out2"][3][:, :DH], invz[:, 1:2], "act")
                chunks.append(norms_b)

                def store_a():
                    nc.sync.dma_start(
                        out_d[b, j][:, 256:384], get_o(u)[:, 256:384]
                    )
                chunks.append(store_a)

                def store_b():
                    nc.sync.dma_start(
                        out_d[b, j][:, 384:512], get_o(u)[:, 384:512]
                    )
                chunks.append(store_b)
                return chunks

            # Software pipeline: scores pieces stream through the psum ring
            # in 512-col steps (s-major within each unit); exp fires per
            # chunk on ACT or DVE; projection chunks of the next unit and
            # PV/normalize/store chunks of completed half-units interleave
            # as fillers.
            from collections import deque

            fillers = deque()
            unit_io = {}

            def unit_inputs(u):
                b, j = units[u]
                yT, ychunks = y_chunks(b, j)
                for c in ychunks:
                    fillers.append(c)
                v_sb, vchunks = v_chunks(b, j)
                for c in vchunks:
                    fillers.append(c)
                unit_io[u] = (yT, v_sb)

            unit_inputs(0)
            u0_chunks = list(fillers)
            fillers.clear()
            for p, (g, w) in enumerate(PIECES):  # scores pieces, s-major
                u = g // 8192
                sh = (g % 8192) // 4096
                tb = (g % 4096) // 512
                so = sh * 512 + (g % 512)  # s-offset within the unit's half
                if p < len(u0_chunks):
                    u0_chunks[p]()  # y chunks before their pieces, then v
                if g % 8192 == 2048 and u + 1 < NU:
                    unit_inputs(u + 1)
                yT, _ = unit_io[u]
                if g % GEN == 0 and g // GEN not in gens:
                    new_gen(g // GEN)
                rp = g % GEN
                nc.tensor.matmul(
                    gens[g // GEN][:, rp:rp + w],
                    fetch_xt(*units[u])[:, tb * 128:(tb + 1) * 128],
                    yT[:, so:so + w],
                    start=True, stop=True,
                )
                c, _ = _chunk_of(g)
                if g + w == int(CSTART[c + 1]):  # chunk complete -> exp
                    emit_exp(c)
                    if c == LAST - 3:
                        # sbl 2,3 park in the tile chunk LAST-2 still reads
                        for ch in last_wave(c, allowed=(0, 1)):
                            fillers.append(ch)
                    elif c in (LAST - 2, LAST - 1):
                        for ch in last_wave(c):
                            fillers.append(ch)
                    elif c == LAST:
                        for ch in last_fin():
                            fillers.append(ch)
                    for hu in range(NHU - 1):
                        if C_END_H[hu] == c:
                            for ch in pv_burst(hu, unit_io[hu // 2][1]):
                                fillers.append(ch)
                for _ in range(2):
                    if fillers:
                        fillers.popleft()()
            while fillers:
                fillers.popleft()()
    if split:
        _split_sync_waits(nc)
    return nc


def _prep_inputs(sequences, Wq, Wk, Wv, bq, bk, bv):
    """Host-side packing: per-core input maps."""
    import ml_dtypes

    sequences = np.ascontiguousarray(np.asarray(sequences, dtype=np.float32))
    Wq = np.asarray(Wq, np.float32)
    Wk = np.asarray(Wk, np.float32)
    Wv = np.asarray(Wv, np.float32)
    bq = np.asarray(bq, np.float32)
    bk = np.asarray(bk, np.float32)
    bv = np.asarray(bv, np.float32)

    # [B, S, H, DH] -> [H, B, DH, S] transposed slices
    xT = np.ascontiguousarray(
        sequences.reshape(B, S, H, DH).transpose(2, 0, 3, 1)
    )  # [H, B, DH, S]

    in_maps = []
    for c in range(NCORES):
        heads = [HPC * c + j for j in range(HPC)]
        xTe = np.zeros((B, HPC, E1, S), np.float32)
        xTe[:, :, DH, :] = 1.0
        for j, h in enumerate(heads):
            xTe[:, j, :DH, :] = xT[h]
        gt = np.zeros((E1, HPC, E1), np.float32)
        wv = np.zeros((E1, HPC, E1), np.float32)
        for j, h in enumerate(heads):
            wq = np.zeros((E1, DH), np.float32)  # x~ -> q, scale folded
            wq[:DH] = Wq[h].T * SCALE
            wq[DH] = bq[h] * SCALE
            wk = np.zeros((E1, DH), np.float32)  # x~ -> k
            wk[:DH] = Wk[h].T
            wk[DH] = bk[h]
            # scores = k.q = x~^T (Wk~ Wq~^T) x~; lhsT of the y-projection
            # is the transpose: G^T = Wq~ @ Wk~^T
            gt[:, j, :] = wq @ wk.T
            wv[:DH, j, :DH] = Wv[h].T
            wv[DH, j, :DH] = bv[h]
            wv[DH, j, DH] = 1.0  # ones column -> Z column of out2
        in_maps.append({
            "xTe": xTe,
            "xbTe": xTe.astype(ml_dtypes.bfloat16),
            "GT": gt.reshape(E1, HPC * E1),
            "WvTe2": wv.reshape(E1, HPC * E1).astype(ml_dtypes.bfloat16),
        })
    return in_maps


def get_nc():
    if "nc" not in _CACHE:
        _CACHE["nc"] = _build_bass()
    return _CACHE["nc"]


def kernel(sequences, Wq, Wk, Wv, bq, bk, bv):
    from concourse.bass_utils import run_bass_kernel_spmd

    nc = get_nc()
    in_maps = _prep_inputs(sequences, Wq, Wk, Wv, bq, bk, bv)
    res = run_bass_kernel_spmd(nc, in_maps, list(range(NCORES)))
    full = np.empty((B, S, D), np.float32)
    for c in range(NCORES):
        # out[b, j, p, blk*64+e] -> full[b, blk*128+p, (2c+j)*64+e]
        arr = res.results[c]["out"].reshape(B, HPC, 128, NT, DH)
        full[:, :, c * HPC * DH:(c + 1) * HPC * DH] = (
            arr.transpose(0, 3, 2, 1, 4).reshape(B, S, HPC * DH)
        )
    return full


# revision 37
# speedup vs baseline: 1.0154x; 1.0047x over previous
"""Multi-head attention (B=4, S=1024, D=1024, H=16, DH=64) on 8 trn2 cores.

Tensor-parallel over heads: core c owns heads {2c, 2c+1}; each core runs
8 independent attention units (4 batches x 2 heads).  Per-head projections
only read a 64-channel slice of the input, so each core receives just its
2x64-channel slice, pre-transposed to [d, s] with a ones-row appended
(E1 = 66: row 64 is the ones row, 65 zero pad).

Math per unit (b, h).  The Wk^T.Wq product is folded on the host
(G^T = Wq~ @ Wk~^T, biases/scale included via the ones-row), so only one
projection feeds the scores:
  y[c,s]    = G^T.T @ xTe           (one 66x66 "projection" replaces q,k)
  scT[t,s]  = xTe.T @ y             (= q.k scores, transposed: t on parts)
  v[t,e']   = xbT.T @ WvTe2         (bf16 inputs: 1 cyc/row; col 64 == 1
                                     -> Z column of out2)
  expT      = exp(scT) -> bf16      (no max-subtraction: |scores| <= ~10)
  out2[s,e']= sum_t expT[t,s] v[t,e']   (transposed PV: s on partitions,
                                     col 64 = Z[s]; per s-block 8
                                     accumulating 66-row bf16 matmuls)
  out[s,e]  = out2[s,e] / Z[s]      (batched 4-way reciprocal + per-sb
                                     tensor_scalar multiply)

Scores stream through a ring of THREE 2-bank PSUM generation tiles
([128, 1024] f32) in 512-col pieces.  exp is split
across TWO engines: ACT chunks use the native Exp; every 4th chunk
(DVE_CHUNKS, 25% of columns) runs on the Vector engine via the
Schraudolph exponent-bit trick -- one tensor_scalar
i16 = round(x * 128/ln2 + B) written through a bf16 bitcast gives
2^(x*log2e) ~ exp(x) with ~3% max element error; since only a quarter
of the keys in each softmax use the approximation the end-to-end
output error stays ~9e-3, under the 2e-2 gate.  The ring depth of 3
is what lets an ACT exp and a DVE exp overlap: pieces of gen g only
wait the exp of gen g-2's chunk.  Engine busy lands near ACT ~50us /
PE ~47us / DVE ~44us instead of the ACT-bound 64us of the
single-engine version (73.3us -> 62.8us end to end).

Pieces are ordered s-major within each unit (s-half, then t-block), so
PV + normalize + store fire per HALF-unit (4 s-blocks, one shared
1-bank psum tile with 4 sequential accumulation groups) ~3 chunks
after the half's scores finish; one strided 4-Z-column reciprocal and
one broadcast tensor_mul normalize 256 output columns per burst.  The
V projection runs on a host-supplied bf16 copy of the input so its
66-col matmuls go 1 cyc/row (f32r pays 4x below 256 cols).  The last
half-unit is staged: sbl 0,1 park open accumulation groups in the two
psM banks (after chunk LAST-3's exp), sbl 2,3 in the retired ring
tile chunk LAST-2 read, so after the final exp only 8 matmuls, 4
normalizes (split ACT/DVE) and 2 narrow stores remain.  A dummy
2-element exp at kernel start pulls the 1.3us activation-table load
into the DMA fill window.
"""

import numpy as np

D = 1024
H = 16
DH = 64
B = 4
S = 1024
NCORES = 8
HPC = H // NCORES  # heads per core = 2
E1 = DH + 2  # 66: ones-row at 64, zero pad at 65
NT = S // 128  # 8 t blocks
NU = B * HPC  # 8 units per core
NHU = NU * 2  # 16 half-units (unit x s-half)
SCALE = 1.0 / np.sqrt(DH)

GEN = 1024  # scores generation: one 2-bank psum tile, in columns
RING = 3  # generations in flight
CHUNKS = [512, 512] + [1024] * 63
N_CH = len(CHUNKS)
LAST = N_CH - 1
assert sum(CHUNKS) == NU * NT * S  # 65536 scores columns per core
CSTART = np.concatenate([[0], np.cumsum(CHUNKS)])
assert all(
    CSTART[i] // GEN == (CSTART[i] + CHUNKS[i] - 1) // GEN
    for i in range(N_CH)
)
# scores pieces: uniform 512 columns (with the PE warmup, finer fill
# pieces no longer pay for their extra instruction overhead)
PIECES = [(512 * k, 512) for k in range(0, 128)]

# chunks whose exp runs on the Vector engine (Schraudolph) instead of ACT:
# every 4th slot offset from the half-unit completion chunks, two extra
# mid-stream slots (16, 32, 48, 56) that rebalance ACT under the warm-PE fill,
# and the FINAL chunk so ACT retires its stream early while the drain
# overlaps the final DVE exp.
DVE_CHUNKS = frozenset(range(2, 59, 4)) | {16, 32, 48, 56, LAST}
A_EXP = 128.0 / np.log(2.0)
B_EXP = 16250.5  # 127*128 - c_opt(~5.5), tuned for hw round-to-nearest

_CACHE = {}


def _chunk_of(g):
    """Chunk index and in-chunk offset for global scores column g."""
    c = int(np.searchsorted(CSTART, g, side="right")) - 1
    return c, g - int(CSTART[c])


def _col(u, sh, tb, sb4=0):
    """Global scores column of unit u, s-half sh, t-block tb, s-subblock."""
    return u * 8192 + sh * 4096 + tb * 512 + sb4 * 128


# chunk whose exp completes each half-unit's scores
C_END_H = [_chunk_of(4096 * (hu + 1) - 1)[0] for hu in range(NHU)]


def _split_sync_waits(nc, limit=1):
    """Walrus in this toolchain rejects instructions carrying more than one
    sync-wait; peel extra waits onto wait-only EventSemaphore ops inserted
    just before, on the same engine queue (engine streams are in-order)."""
    import concourse.mybir as mybir

    n = 0
    for bb in nc.main_func.blocks:
        out = []
        for ins in bb.instructions:
            si = ins.sync_info
            if si is not None and len(si.on_wait) > limit:
                waits = list(si.on_wait)
                for w in waits[:-limit]:
                    ev = mybir.InstEventSemaphore(
                        name=f"WSPLIT-{n}", ins=[], outs=[]
                    )
                    n += 1
                    ev.engine = ins.engine
                    ev.sync_info = mybir.SyncInfo(on_wait=[w], on_update=[])
                    out.append(ev)
                ins.sync_info = mybir.SyncInfo(
                    on_wait=waits[-limit:], on_update=list(si.on_update)
                )
            out.append(ins)
        bb.instructions = out
    return n


def _build_bass(split=True):
    import concourse.bass as bass
    import concourse.mybir as mybir
    import concourse.tile as tile

    f32 = mybir.dt.float32
    f32r = mybir.dt.float32r
    bf16 = mybir.dt.bfloat16
    i16 = mybir.dt.int16
    nc = bass.Bass()

    xTe_d = nc.declare_dram_parameter("xTe", [B, HPC, E1, S], f32r, isOutput=False)
    xb_d = nc.declare_dram_parameter("xbTe", [B, HPC, E1, S], bf16, isOutput=False)
    gt_d = nc.declare_dram_parameter("GT", [E1, HPC * E1], f32r, isOutput=False)
    wv_d = nc.declare_dram_parameter("WvTe2", [E1, HPC * E1], bf16, isOutput=False)
    # out[b, j, p, blk*64 + e] == attention(b, s=blk*128+p, head j)[e]
    out_d = nc.declare_dram_parameter("out", [B, HPC, 128, 512], f32, isOutput=True)

    with tile.TileContext(nc) as tc:
        with (
            tc.tile_pool(name="const", bufs=1) as constp,
            tc.tile_pool(name="sb", bufs=2) as sbp,
            tc.tile_pool(name="expp", bufs=2) as expp,
            tc.tile_pool(name="psR", bufs=1, space="PSUM") as psR,
            tc.tile_pool(name="psM", bufs=2, space="PSUM") as psM,
        ):
            # dummy activation: pulls the exp table load into the DMA fill
            dummy = constp.tile([1, 4], f32, name="dummy")
            nc.gpsimd.memset(dummy[:], 0.0)
            nc.scalar.activation(
                dummy[:, 2:4], dummy[:, 0:2], mybir.ActivationFunctionType.Exp
            )

            gt_sb = constp.tile([E1, HPC * E1], f32r)
            wv_sb = constp.tile([E1, HPC * E1], bf16)
            nc.gpsimd.dma_start(gt_sb[:], gt_d[:])
            nc.gpsimd.dma_start(wv_sb[:], wv_d[:])

            gens = {}  # generation idx -> ring scores tile

            def new_gen(i):
                for k in range(max(gens, default=-1) + 1, i + 1):
                    gens[k] = psR.tile([128, GEN], f32, tag="sc", bufs=RING,
                                       name=f"sc_{k}")
                return gens[i]

            # PE warmup: a 2-col matmul at t~0 so the fill-phase projection
            # matmuls run at ramped pstate instead of cold
            nc.tensor.matmul(
                new_gen(0)[:2, 0:2], dummy[:, 0:2], dummy[:, 2:4],
                start=True, stop=True,
            )

            units = [(b, j) for b in range(B) for j in range(HPC)]

            xts = {}
            xbs = {}

            def fetch_xt(b, j):
                if (b, j) not in xts:
                    for jj in range(HPC):
                        xts[(b, jj)] = sbp.tile(
                            [E1, S], f32r, tag="xt", bufs=4,
                            name=f"xt_{b}_{jj}",
                        )
                        xbs[(b, jj)] = sbp.tile(
                            [E1, S], bf16, tag="xb", bufs=4,
                            name=f"xb_{b}_{jj}",
                        )
                    for jj in range(HPC):  # j-major: head 0 complete first
                        # finer first transfer for unit 0: the first y chunk
                        # and scores piece only need 256 columns
                        spans = [(0, 512), (512, 512)]
                        for s0, w in spans:
                            nc.sync.dma_start(
                                xts[(b, jj)][:, s0:s0 + w],
                                xTe_d[b, jj, :, s0:s0 + w],
                            )
                    for jj in range(HPC):
                        nc.sync.dma_start(xbs[(b, jj)][:], xb_d[b, jj])
                return xts[(b, j)]

            def y_chunks(b, j, fine=False):
                """Scores projection y = G^T.T @ x as filler chunks."""
                xt = fetch_xt(b, j)
                yT = sbp.tile([E1, S], f32r, tag="yT", bufs=3, name=f"y_{b}_{j}")
                chunks = []
                spans = ([(0, 256), (256, 256), (512, 512)] if fine
                         else [(0, 512), (512, 512)])
                for s0, w in spans:
                    def chunk(s0=s0, w=w):
                        y_ps = psM.tile(
                            [128, 512], f32, tag="m", bufs=2, name="y_ps"
                        )
                        nc.tensor.matmul(
                            y_ps[:E1, :w],
                            gt_sb[:, j * E1:(j + 1) * E1],
                            xt[:, s0:s0 + w],
                            start=True, stop=True,
                        )
                        nc.vector.tensor_copy(yT[:, s0:s0 + w], y_ps[:E1, :w])
                    chunks.append(chunk)
                return yT, chunks

            def v_chunks(b, j):
                """v projection as 2 filler chunks (4 bf16 MMs + copy each).

                bf16 x bf16 matmuls run 1 cyc/row even at 66-col outputs
                (f32r would pay 4x below 256 cols), fed by the host-packed
                bf16 copy of the input slice.
                """
                fetch_xt(b, j)
                xb = xbs[(b, j)]
                v_sb = sbp.tile(
                    [128, NT * E1], bf16, tag="v", bufs=4, name=f"v_{b}_{j}"
                )
                chunks = []
                for half in range(2):
                    def chunk(half=half):
                        v_ps = psM.tile(
                            [128, 512], f32, tag="m", bufs=2, name="v_ps"
                        )
                        for q in range(4):
                            tb = half * 4 + q
                            nc.tensor.matmul(
                                v_ps[:, q * E1:(q + 1) * E1],
                                xb[:, tb * 128:(tb + 1) * 128],
                                wv_sb[:, j * E1:(j + 1) * E1],
                                start=True, stop=True,
                            )
                        nc.vector.tensor_copy(
                            v_sb[:, half * 4 * E1:(half + 1) * 4 * E1],
                            v_ps[:, :4 * E1],
                        )
                    chunks.append(chunk)
                return v_sb, chunks

            chunk_exp = {}  # chunk idx -> expT tile

            def emit_exp(c):
                """exp of completed chunk c: ACT Exp, or DVE Schraudolph."""
                csz = CHUNKS[c]
                cp = int(CSTART[c]) % GEN
                src = gens[int(CSTART[c]) // GEN]
                expT = expp.tile(
                    [128, GEN], bf16, tag="expT", bufs=16, name="expT"
                )
                if c in DVE_CHUNKS:
                    nc.vector.tensor_scalar(
                        expT[:, :csz].bitcast(i16),
                        src[:, cp:cp + csz],
                        float(A_EXP),
                        float(B_EXP),
                        mybir.AluOpType.mult,
                        mybir.AluOpType.add,
                    )
                else:
                    nc.scalar.activation(
                        expT[:, :csz], src[:, cp:cp + csz],
                        mybir.ActivationFunctionType.Exp,
                    )
                chunk_exp[c] = expT

            o_sbs = {}

            def get_o(u):
                if u not in o_sbs:
                    b, j = units[u]
                    o_sbs[u] = sbp.tile([128, 512], f32, tag="o", bufs=2,
                                        name=f"o_{b}_{j}")
                return o_sbs[u]

            def pv_mms(u, sh, sbl, out2, v_sb, tbs, first):
                for tb in tbs:
                    c, off = _chunk_of(_col(u, sh, tb, sbl))
                    nc.tensor.matmul(
                        out2,
                        chunk_exp[c][:, off:off + 128],
                        v_sb[:, tb * E1:(tb + 1) * E1],
                        start=(tb == tbs[0] and first),
                        stop=(tb == NT - 1),
                    )

            def scale_sb(u, sh, sbl, out2_64, invz_col, eng="dve"):
                o_sl = get_o(u)[:, (sh * 4 + sbl) * DH:(sh * 4 + sbl + 1) * DH]
                if eng == "act":
                    nc.scalar.activation(
                        o_sl, out2_64,
                        mybir.ActivationFunctionType.Copy,
                        scale=invz_col,
                    )
                else:
                    nc.vector.tensor_scalar_mul(o_sl, out2_64, invz_col)

            def pv_burst(hu, v_sb):
                """PV + normalize + store for half-unit hu (4 s-blocks).

                All four out2 regions live in ONE 1-bank psum tile (the
                accumulation groups run sequentially), so a single strided
                reciprocal covers the four Z columns."""
                u, sh = hu // 2, hu % 2
                shared = {}
                chunks = []
                for sbl in range(4):
                    def chunk(sbl=sbl):
                        if "out2" not in shared:
                            shared["out2"] = psM.tile(
                                [128, 512], f32, tag="m", bufs=2,
                                name=f"pv_{hu}",
                            )
                        out2 = shared["out2"]
                        pv_mms(u, sh, sbl, out2[:, sbl * E1:sbl * E1 + E1],
                               v_sb, list(range(NT)), True)
                    chunks.append(chunk)

                def norms():
                    out2 = shared["out2"]
                    invz = sbp.tile([128, 4], f32, tag="invz", bufs=8,
                                    name="invz")
                    nc.vector.reciprocal(
                        invz[:], out2[:, DH:3 * E1 + DH + 1:E1]
                    )
                    # one 256-col multiply for all 4 s-blocks: the strided
                    # out2 view picks the 64 value cols of each block and
                    # invz broadcasts along the inner dim.
                    o = get_o(u)[:, sh * 256:(sh + 1) * 256]
                    nc.vector.tensor_mul(
                        o.rearrange("p (b e) -> p b e", e=DH),
                        out2[:, :4 * E1].rearrange(
                            "p (b e) -> p b e", e=E1)[:, :, :DH],
                        invz[:].rearrange("p (b e) -> p b e", e=1)
                        .broadcast_to([128, 4, DH]),
                    )
                chunks.append(norms)

                def store():
                    nc.sync.dma_start(
                        out_d[units[u][0], units[u][1]][:, sh * 256:(sh + 1) * 256],
                        get_o(u)[:, sh * 256:(sh + 1) * 256],
                    )
                chunks.append(store)
                return chunks

            # --- last half-unit (hu = 15): staged so almost nothing trails
            # the final exp.  sbl 0,1 park open accumulation groups in the
            # two psM banks (after chunk LAST-3's exp); sbl 2,3 in the two
            # banks of the retired gen tile that chunk LAST-2 read.  After
            # the final exp: 2 matmuls per s-block, 4 normalizes
            # (alternating DVE / ACT), 2 narrow stores.
            last_state = {}

            def last_wave(ceil_c, allowed=(0, 1, 2, 3)):
                u, sh = NU - 1, 1
                v_sb = unit_io[u][1]
                st = last_state
                if "done" not in st:
                    st["done"] = [0] * 4
                    st["out2"] = {}
                    st["started"] = [False] * 4
                chunks = []
                for sbl in allowed:
                    tbs = [tb for tb in range(NT)
                           if st["done"][sbl] <= tb
                           and _chunk_of(_col(u, sh, tb, sbl))[0] <= ceil_c]
                    if not tbs:
                        continue
                    if sbl not in st["out2"]:
                        if sbl < 2:
                            t = psM.tile([128, 512], f32, tag="m", bufs=2,
                                         name=f"lpv_{sbl}")
                            st["out2"][sbl] = t[:, :E1]
                        else:
                            # the tile chunk LAST-2 read is retired once that
                            # exp ran (the final chunk lives in a different
                            # ring slot) -- park sbl 2,3 in its two banks.
                            gi = int(CSTART[LAST - 2]) // GEN
                            off = (sbl - 2) * 512
                            st["out2"][sbl] = new_gen(gi)[:, off:off + E1]
                    out2 = st["out2"][sbl]
                    first = not st["started"][sbl]
                    st["started"][sbl] = True
                    st["done"][sbl] = tbs[-1] + 1

                    def ch(sbl=sbl, tbs=tbs, out2=out2, first=first):
                        pv_mms(u, sh, sbl, out2, v_sb, tbs, first)
                    chunks.append(ch)
                return chunks

            def last_fin():
                u, sh = NU - 1, 1
                b, j = units[u]
                chunks = list(last_wave(LAST))

                def norms_a():
                    st = last_state
                    invz = sbp.tile([128, 2], f32, tag="invz", bufs=8,
                                    name="invz")
                    nc.vector.reciprocal(invz[:, 0:1], st["out2"][0][:, DH:DH + 1])
                    nc.vector.reciprocal(invz[:, 1:2], st["out2"][1][:, DH:DH + 1])
                    scale_sb(u, sh, 0, st["out2"][0][:, :DH], invz[:, 0:1], "dve")
                    scale_sb(u, sh, 1, st["out2"][1][:, :DH], invz[:, 1:2], "act")
                chunks.append(norms_a)

                def norms_b():
                    st = last_state
                    invz = sbp.tile([128, 2], f32, tag="invz", bufs=8,
                                    name="invz")
                    # sbl 2,3 live in the same ring tile: one strided recip
                    g61 = new_gen(int(CSTART[LAST - 2]) // GEN)
                    nc.vector.reciprocal(invz[:], g61[:, DH:DH + 513:512])
                    scale_sb(u, sh, 2, st["out2"][2][:, :DH], invz[:, 0:1], "dve")
                    scale_sb(u, sh, 3, st["out2"][3][:, :DH], invz[:, 1:2], "act")
                chunks.append(norms_b)

                def store_a():
                    nc.sync.dma_start(
                        out_d[b, j][:, 256:384], get_o(u)[:, 256:384]
                    )
                chunks.append(store_a)

                def store_b():
                    nc.sync.dma_start(
                        out_d[b, j][:, 384:512], get_o(u)[:, 384:512]
                    )
                chunks.append(store_b)
                return chunks

            # Software pipeline: scores pieces stream through the psum ring
            # in 512-col steps (s-major within each unit); exp fires per
            # chunk on ACT or DVE; projection chunks of the next unit and
            # PV/normalize/store chunks of completed half-units interleave
            # as fillers.
            from collections import deque

            fillers = deque()
            unit_io = {}

            def unit_inputs(u):
                b, j = units[u]
                yT, ychunks = y_chunks(b, j)
                for c in ychunks:
                    fillers.append(c)
                v_sb, vchunks = v_chunks(b, j)
                for c in vchunks:
                    fillers.append(c)
                unit_io[u] = (yT, v_sb)

            unit_inputs(0)
            u0_chunks = list(fillers)
            fillers.clear()
            for p, (g, w) in enumerate(PIECES):  # scores pieces, s-major
                u = g // 8192
                sh = (g % 8192) // 4096
                tb = (g % 4096) // 512
                so = sh * 512 + (g % 512)  # s-offset within the unit's half
                if p < len(u0_chunks):
                    u0_chunks[p]()  # y chunks before their pieces, then v
                if g % 8192 == 2048 and u + 1 < NU:
                    unit_inputs(u + 1)
                yT, _ = unit_io[u]
                if g % GEN == 0 and g // GEN not in gens:
                    new_gen(g // GEN)
                rp = g % GEN
                nc.tensor.matmul(
                    gens[g // GEN][:, rp:rp + w],
                    fetch_xt(*units[u])[:, tb * 128:(tb + 1) * 128],
                    yT[:, so:so + w],
                    start=True, stop=True,
                )
                c, _ = _chunk_of(g)
                if g + w == int(CSTART[c + 1]):  # chunk complete -> exp
                    emit_exp(c)
                    if c == LAST - 3:
                        # sbl 2,3 park in the tile chunk LAST-2 still reads
                        for ch in last_wave(c, allowed=(0, 1)):
                            fillers.append(ch)
                    elif c in (LAST - 2, LAST - 1):
                        for ch in last_wave(c):
                            fillers.append(ch)
                    elif c == LAST:
                        for ch in last_fin():
                            fillers.append(ch)
                    for hu in range(NHU - 1):
                        if C_END_H[hu] == c:
                            for ch in pv_burst(hu, unit_io[hu // 2][1]):
                                fillers.append(ch)
                for _ in range(2):
                    if fillers:
                        fillers.popleft()()
            while fillers:
                fillers.popleft()()
    if split:
        _split_sync_waits(nc)
    return nc


def _prep_inputs(sequences, Wq, Wk, Wv, bq, bk, bv):
    """Host-side packing: per-core input maps."""
    import ml_dtypes

    sequences = np.ascontiguousarray(np.asarray(sequences, dtype=np.float32))
    Wq = np.asarray(Wq, np.float32)
    Wk = np.asarray(Wk, np.float32)
    Wv = np.asarray(Wv, np.float32)
    bq = np.asarray(bq, np.float32)
    bk = np.asarray(bk, np.float32)
    bv = np.asarray(bv, np.float32)

    # [B, S, H, DH] -> [H, B, DH, S] transposed slices
    xT = np.ascontiguousarray(
        sequences.reshape(B, S, H, DH).transpose(2, 0, 3, 1)
    )  # [H, B, DH, S]

    in_maps = []
    for c in range(NCORES):
        heads = [HPC * c + j for j in range(HPC)]
        xTe = np.zeros((B, HPC, E1, S), np.float32)
        xTe[:, :, DH, :] = 1.0
        for j, h in enumerate(heads):
            xTe[:, j, :DH, :] = xT[h]
        gt = np.zeros((E1, HPC, E1), np.float32)
        wv = np.zeros((E1, HPC, E1), np.float32)
        for j, h in enumerate(heads):
            wq = np.zeros((E1, DH), np.float32)  # x~ -> q, scale folded
            wq[:DH] = Wq[h].T * SCALE
            wq[DH] = bq[h] * SCALE
            wk = np.zeros((E1, DH), np.float32)  # x~ -> k
            wk[:DH] = Wk[h].T
            wk[DH] = bk[h]
            # scores = k.q = x~^T (Wk~ Wq~^T) x~; lhsT of the y-projection
            # is the transpose: G^T = Wq~ @ Wk~^T
            gt[:, j, :] = wq @ wk.T
            wv[:DH, j, :DH] = Wv[h].T
            wv[DH, j, :DH] = bv[h]
            wv[DH, j, DH] = 1.0  # ones column -> Z column of out2
        in_maps.append({
            "xTe": xTe,
            "xbTe": xTe.astype(ml_dtypes.bfloat16),
            "GT": gt.reshape(E1, HPC * E1),
            "WvTe2": wv.reshape(E1, HPC * E1).astype(ml_dtypes.bfloat16),
        })
    return in_maps


def get_nc():
    if "nc" not in _CACHE:
        _CACHE["nc"] = _build_bass()
    return _CACHE["nc"]


def kernel(sequences, Wq, Wk, Wv, bq, bk, bv):
    from concourse.bass_utils import run_bass_kernel_spmd

    nc = get_nc()
    in_maps = _prep_inputs(sequences, Wq, Wk, Wv, bq, bk, bv)
    res = run_bass_kernel_spmd(nc, in_maps, list(range(NCORES)))
    full = np.empty((B, S, D), np.float32)
    for c in range(NCORES):
        # out[b, j, p, blk*64+e] -> full[b, blk*128+p, (2c+j)*64+e]
        arr = res.results[c]["out"].reshape(B, HPC, 128, NT, DH)
        full[:, :, c * HPC * DH:(c + 1) * HPC * DH] = (
            arr.transpose(0, 3, 2, 1, 4).reshape(B, S, HPC * DH)
        )
    return full


# revision 38
# speedup vs baseline: 1.0289x; 1.0133x over previous
"""Multi-head attention (B=4, S=1024, D=1024, H=16, DH=64) on 8 trn2 cores.

Tensor-parallel over heads: core c owns heads {2c, 2c+1}; each core runs
8 independent attention units (4 batches x 2 heads).  Per-head projections
only read a 64-channel slice of the input, so each core receives just its
2x64-channel slice, pre-transposed to [d, s] with a ones-row appended
(E1 = 66: row 64 is the ones row, 65 zero pad).

Math per unit (b, h).  The Wk^T.Wq product is folded on the host
(G^T = Wq~ @ Wk~^T, biases/scale included via the ones-row), so only one
projection feeds the scores:
  y[c,s]    = G^T.T @ xTe           (one 66x66 "projection" replaces q,k)
  scT[t,s]  = xTe.T @ y             (= q.k scores, transposed: t on parts)
  v[t,e']   = xbT.T @ WvTe2         (bf16 inputs: 1 cyc/row; col 64 == 1
                                     -> Z column of out2)
  expT      = exp(scT) -> bf16      (no max-subtraction: |scores| <= ~10)
  out2[s,e']= sum_t expT[t,s] v[t,e']   (transposed PV: s on partitions,
                                     col 64 = Z[s]; per s-block 8
                                     accumulating 66-row bf16 matmuls)
  out[s,e]  = out2[s,e] / Z[s]      (batched 4-way reciprocal + per-sb
                                     tensor_scalar multiply)

Scores stream through a ring of THREE 2-bank PSUM generation tiles
([128, 1024] f32) in 512-col pieces.  exp is split
across TWO engines: ACT chunks use the native Exp; every 4th chunk
(DVE_CHUNKS, 25% of columns) runs on the Vector engine via the
Schraudolph exponent-bit trick -- one tensor_scalar
i16 = round(x * 128/ln2 + B) written through a bf16 bitcast gives
2^(x*log2e) ~ exp(x) with ~3% max element error; since only a quarter
of the keys in each softmax use the approximation the end-to-end
output error stays ~9e-3, under the 2e-2 gate.  The ring depth of 3
is what lets an ACT exp and a DVE exp overlap: pieces of gen g only
wait the exp of gen g-2's chunk.  Engine busy lands near ACT ~50us /
PE ~47us / DVE ~44us instead of the ACT-bound 64us of the
single-engine version (73.3us -> 62.8us end to end).

Pieces are ordered s-major within each unit (s-half, then t-block), so
PV + normalize + store fire per HALF-unit (4 s-blocks, one shared
1-bank psum tile with 4 sequential accumulation groups) ~3 chunks
after the half's scores finish; one strided 4-Z-column reciprocal and
one broadcast tensor_mul normalize 256 output columns per burst.  The
V projection runs on a host-supplied bf16 copy of the input so its
66-col matmuls go 1 cyc/row (f32r pays 4x below 256 cols).  The last
half-unit is staged: sbl 0,1 park open accumulation groups in the two
psM banks (after chunk LAST-3's exp), sbl 2,3 in the retired ring
tile chunk LAST-2 read, so after the final exp only 8 matmuls, 4
normalizes (split ACT/DVE) and 2 narrow stores remain.  A dummy
2-element exp at kernel start pulls the 1.3us activation-table load
into the DMA fill window.
"""

import numpy as np

D = 1024
H = 16
DH = 64
B = 4
S = 1024
NCORES = 8
HPC = H // NCORES  # heads per core = 2
E1 = DH + 2  # 66: ones-row at 64, zero pad at 65
NT = S // 128  # 8 t blocks
NU = B * HPC  # 8 units per core
NHU = NU * 2  # 16 half-units (unit x s-half)
SCALE = 1.0 / np.sqrt(DH)

GEN = 1024  # scores generation: one 2-bank psum tile, in columns
RING = 3  # generations in flight
CHUNKS = [512, 512] + [1024] * 63
N_CH = len(CHUNKS)
LAST = N_CH - 1
assert sum(CHUNKS) == NU * NT * S  # 65536 scores columns per core
CSTART = np.concatenate([[0], np.cumsum(CHUNKS)])
assert all(
    CSTART[i] // GEN == (CSTART[i] + CHUNKS[i] - 1) // GEN
    for i in range(N_CH)
)
# scores pieces: uniform 512 columns (with the PE warmup, finer fill
# pieces no longer pay for their extra instruction overhead)
PIECES = [(512 * k, 512) for k in range(0, 128)]

# chunks whose exp runs on the Vector engine (Schraudolph) instead of ACT:
# every 4th slot offset from the half-unit completion chunks, two extra
# extra slots (16, 32, 48, 56, 60) that rebalance ACT under the warm-PE fill,
# and the FINAL chunk so ACT retires its stream early while the drain
# overlaps the final DVE exp.
DVE_CHUNKS = frozenset(range(2, 59, 4)) | {16, 32, 48, 56, 60, LAST}
A_EXP = 128.0 / np.log(2.0)
B_EXP = 16250.5  # 127*128 - c_opt(~5.5), tuned for hw round-to-nearest

_CACHE = {}


def _chunk_of(g):
    """Chunk index and in-chunk offset for global scores column g."""
    c = int(np.searchsorted(CSTART, g, side="right")) - 1
    return c, g - int(CSTART[c])


def _col(u, sh, tb, sb4=0):
    """Global scores column of unit u, s-half sh, t-block tb, s-subblock."""
    return u * 8192 + sh * 4096 + tb * 512 + sb4 * 128


# chunk whose exp completes each half-unit's scores
C_END_H = [_chunk_of(4096 * (hu + 1) - 1)[0] for hu in range(NHU)]


def _split_sync_waits(nc, limit=1):
    """Walrus in this toolchain rejects instructions carrying more than one
    sync-wait; peel extra waits onto wait-only EventSemaphore ops inserted
    just before, on the same engine queue (engine streams are in-order)."""
    import concourse.mybir as mybir

    n = 0
    for bb in nc.main_func.blocks:
        out = []
        for ins in bb.instructions:
            si = ins.sync_info
            if si is not None and len(si.on_wait) > limit:
                waits = list(si.on_wait)
                for w in waits[:-limit]:
                    ev = mybir.InstEventSemaphore(
                        name=f"WSPLIT-{n}", ins=[], outs=[]
                    )
                    n += 1
                    ev.engine = ins.engine
                    ev.sync_info = mybir.SyncInfo(on_wait=[w], on_update=[])
                    out.append(ev)
                ins.sync_info = mybir.SyncInfo(
                    on_wait=waits[-limit:], on_update=list(si.on_update)
                )
            out.append(ins)
        bb.instructions = out
    return n


def _build_bass(split=True):
    import concourse.bass as bass
    import concourse.mybir as mybir
    import concourse.tile as tile

    f32 = mybir.dt.float32
    f32r = mybir.dt.float32r
    bf16 = mybir.dt.bfloat16
    i16 = mybir.dt.int16
    nc = bass.Bass()

    xTe_d = nc.declare_dram_parameter("xTe", [B, HPC, E1, S], f32r, isOutput=False)
    xb_d = nc.declare_dram_parameter("xbTe", [B, HPC, E1, S], bf16, isOutput=False)
    gt_d = nc.declare_dram_parameter("GT", [E1, HPC * E1], f32r, isOutput=False)
    wv_d = nc.declare_dram_parameter("WvTe2", [E1, HPC * E1], bf16, isOutput=False)
    # out[b, j, p, blk*64 + e] == attention(b, s=blk*128+p, head j)[e]
    out_d = nc.declare_dram_parameter("out", [B, HPC, 128, 512], f32, isOutput=True)

    with tile.TileContext(nc) as tc:
        with (
            tc.tile_pool(name="const", bufs=1) as constp,
            tc.tile_pool(name="sb", bufs=2) as sbp,
            tc.tile_pool(name="expp", bufs=2) as expp,
            tc.tile_pool(name="psR", bufs=1, space="PSUM") as psR,
            tc.tile_pool(name="psM", bufs=2, space="PSUM") as psM,
        ):
            # dummy activation: pulls the exp table load into the DMA fill
            dummy = constp.tile([1, 4], f32, name="dummy")
            nc.gpsimd.memset(dummy[:], 0.0)
            nc.scalar.activation(
                dummy[:, 2:4], dummy[:, 0:2], mybir.ActivationFunctionType.Exp
            )

            gt_sb = constp.tile([E1, HPC * E1], f32r)
            wv_sb = constp.tile([E1, HPC * E1], bf16)
            nc.gpsimd.dma_start(gt_sb[:], gt_d[:])
            nc.gpsimd.dma_start(wv_sb[:], wv_d[:])

            gens = {}  # generation idx -> ring scores tile

            def new_gen(i):
                for k in range(max(gens, default=-1) + 1, i + 1):
                    gens[k] = psR.tile([128, GEN], f32, tag="sc", bufs=RING,
                                       name=f"sc_{k}")
                return gens[i]

            # PE warmup: a 2-col matmul at t~0 so the fill-phase projection
            # matmuls run at ramped pstate instead of cold
            nc.tensor.matmul(
                new_gen(0)[:2, 0:2], dummy[:, 0:2], dummy[:, 2:4],
                start=True, stop=True,
            )

            units = [(b, j) for b in range(B) for j in range(HPC)]

            xts = {}
            xbs = {}

            def fetch_xt(b, j):
                if (b, j) not in xts:
                    for jj in range(HPC):
                        xts[(b, jj)] = sbp.tile(
                            [E1, S], f32r, tag="xt", bufs=4,
                            name=f"xt_{b}_{jj}",
                        )
                        xbs[(b, jj)] = sbp.tile(
                            [E1, S], bf16, tag="xb", bufs=4,
                            name=f"xb_{b}_{jj}",
                        )
                    for jj in range(HPC):  # j-major: head 0 complete first
                        # finer first transfer for unit 0: the first y chunk
                        # and scores piece only need 256 columns
                        spans = [(0, 512), (512, 512)]
                        for s0, w in spans:
                            nc.sync.dma_start(
                                xts[(b, jj)][:, s0:s0 + w],
                                xTe_d[b, jj, :, s0:s0 + w],
                            )
                    for jj in range(HPC):
                        nc.sync.dma_start(xbs[(b, jj)][:], xb_d[b, jj])
                return xts[(b, j)]

            def y_chunks(b, j, fine=False):
                """Scores projection y = G^T.T @ x as filler chunks."""
                xt = fetch_xt(b, j)
                yT = sbp.tile([E1, S], f32r, tag="yT", bufs=3, name=f"y_{b}_{j}")
                chunks = []
                spans = ([(0, 256), (256, 256), (512, 512)] if fine
                         else [(0, 512), (512, 512)])
                for s0, w in spans:
                    def chunk(s0=s0, w=w):
                        y_ps = psM.tile(
                            [128, 512], f32, tag="m", bufs=2, name="y_ps"
                        )
                        nc.tensor.matmul(
                            y_ps[:E1, :w],
                            gt_sb[:, j * E1:(j + 1) * E1],
                            xt[:, s0:s0 + w],
                            start=True, stop=True,
                        )
                        nc.vector.tensor_copy(yT[:, s0:s0 + w], y_ps[:E1, :w])
                    chunks.append(chunk)
                return yT, chunks

            def v_chunks(b, j):
                """v projection as 2 filler chunks (4 bf16 MMs + copy each).

                bf16 x bf16 matmuls run 1 cyc/row even at 66-col outputs
                (f32r would pay 4x below 256 cols), fed by the host-packed
                bf16 copy of the input slice.
                """
                fetch_xt(b, j)
                xb = xbs[(b, j)]
                v_sb = sbp.tile(
                    [128, NT * E1], bf16, tag="v", bufs=4, name=f"v_{b}_{j}"
                )
                chunks = []
                for half in range(2):
                    def chunk(half=half):
                        v_ps = psM.tile(
                            [128, 512], f32, tag="m", bufs=2, name="v_ps"
                        )
                        for q in range(4):
                            tb = half * 4 + q
                            nc.tensor.matmul(
                                v_ps[:, q * E1:(q + 1) * E1],
                                xb[:, tb * 128:(tb + 1) * 128],
                                wv_sb[:, j * E1:(j + 1) * E1],
                                start=True, stop=True,
                            )
                        nc.vector.tensor_copy(
                            v_sb[:, half * 4 * E1:(half + 1) * 4 * E1],
                            v_ps[:, :4 * E1],
                        )
                    chunks.append(chunk)
                return v_sb, chunks

            chunk_exp = {}  # chunk idx -> expT tile

            def emit_exp(c):
                """exp of completed chunk c: ACT Exp, or DVE Schraudolph."""
                csz = CHUNKS[c]
                cp = int(CSTART[c]) % GEN
                src = gens[int(CSTART[c]) // GEN]
                expT = expp.tile(
                    [128, GEN], bf16, tag="expT", bufs=16, name="expT"
                )
                if c in DVE_CHUNKS:
                    nc.vector.tensor_scalar(
                        expT[:, :csz].bitcast(i16),
                        src[:, cp:cp + csz],
                        float(A_EXP),
                        float(B_EXP),
                        mybir.AluOpType.mult,
                        mybir.AluOpType.add,
                    )
                else:
                    nc.scalar.activation(
                        expT[:, :csz], src[:, cp:cp + csz],
                        mybir.ActivationFunctionType.Exp,
                    )
                chunk_exp[c] = expT

            o_sbs = {}

            def get_o(u):
                if u not in o_sbs:
                    b, j = units[u]
                    o_sbs[u] = sbp.tile([128, 512], f32, tag="o", bufs=2,
                                        name=f"o_{b}_{j}")
                return o_sbs[u]

            def pv_mms(u, sh, sbl, out2, v_sb, tbs, first):
                for tb in tbs:
                    c, off = _chunk_of(_col(u, sh, tb, sbl))
                    nc.tensor.matmul(
                        out2,
                        chunk_exp[c][:, off:off + 128],
                        v_sb[:, tb * E1:(tb + 1) * E1],
                        start=(tb == tbs[0] and first),
                        stop=(tb == NT - 1),
                    )

            def scale_sb(u, sh, sbl, out2_64, invz_col, eng="dve"):
                o_sl = get_o(u)[:, (sh * 4 + sbl) * DH:(sh * 4 + sbl + 1) * DH]
                if eng == "act":
                    nc.scalar.activation(
                        o_sl, out2_64,
                        mybir.ActivationFunctionType.Copy,
                        scale=invz_col,
                    )
                else:
                    nc.vector.tensor_scalar_mul(o_sl, out2_64, invz_col)

            def pv_burst(hu, v_sb):
                """PV + normalize + store for half-unit hu (4 s-blocks).

                All four out2 regions live in ONE 1-bank psum tile (the
                accumulation groups run sequentially), so a single strided
                reciprocal covers the four Z columns."""
                u, sh = hu // 2, hu % 2
                shared = {}
                chunks = []
                for sbl in range(4):
                    def chunk(sbl=sbl):
                        if "out2" not in shared:
                            shared["out2"] = psM.tile(
                                [128, 512], f32, tag="m", bufs=2,
                                name=f"pv_{hu}",
                            )
                        out2 = shared["out2"]
                        pv_mms(u, sh, sbl, out2[:, sbl * E1:sbl * E1 + E1],
                               v_sb, list(range(NT)), True)
                    chunks.append(chunk)

                def norms():
                    out2 = shared["out2"]
                    invz = sbp.tile([128, 4], f32, tag="invz", bufs=8,
                                    name="invz")
                    nc.vector.reciprocal(
                        invz[:], out2[:, DH:3 * E1 + DH + 1:E1]
                    )
                    # one 256-col multiply for all 4 s-blocks: the strided
                    # out2 view picks the 64 value cols of each block and
                    # invz broadcasts along the inner dim.
                    o = get_o(u)[:, sh * 256:(sh + 1) * 256]
                    nc.vector.tensor_mul(
                        o.rearrange("p (b e) -> p b e", e=DH),
                        out2[:, :4 * E1].rearrange(
                            "p (b e) -> p b e", e=E1)[:, :, :DH],
                        invz[:].rearrange("p (b e) -> p b e", e=1)
                        .broadcast_to([128, 4, DH]),
                    )
                chunks.append(norms)

                def store():
                    nc.sync.dma_start(
                        out_d[units[u][0], units[u][1]][:, sh * 256:(sh + 1) * 256],
                        get_o(u)[:, sh * 256:(sh + 1) * 256],
                    )
                chunks.append(store)
                return chunks

            # --- last half-unit (hu = 15): staged so almost nothing trails
            # the final exp.  sbl 0,1 park open accumulation groups in the
            # two psM banks (after chunk LAST-3's exp); sbl 2,3 in the two
            # banks of the retired gen tile that chunk LAST-2 read.  After
            # the final exp: 2 matmuls per s-block, 4 normalizes
            # (alternating DVE / ACT), 2 narrow stores.
            last_state = {}

            def last_wave(ceil_c, allowed=(0, 1, 2, 3)):
                u, sh = NU - 1, 1
                v_sb = unit_io[u][1]
                st = last_state
                if "done" not in st:
                    st["done"] = [0] * 4
                    st["out2"] = {}
                    st["started"] = [False] * 4
                chunks = []
                for sbl in allowed:
                    tbs = [tb for tb in range(NT)
                           if st["done"][sbl] <= tb
                           and _chunk_of(_col(u, sh, tb, sbl))[0] <= ceil_c]
                    if not tbs:
                        continue
                    if sbl not in st["out2"]:
                        if sbl < 2:
                            t = psM.tile([128, 512], f32, tag="m", bufs=2,
                                         name=f"lpv_{sbl}")
                            st["out2"][sbl] = t[:, :E1]
                        else:
                            # the tile chunk LAST-2 read is retired once that
                            # exp ran (the final chunk lives in a different
                            # ring slot) -- park sbl 2,3 in its two banks.
                            gi = int(CSTART[LAST - 2]) // GEN
                            off = (sbl - 2) * 512
                            st["out2"][sbl] = new_gen(gi)[:, off:off + E1]
                    out2 = st["out2"][sbl]
                    first = not st["started"][sbl]
                    st["started"][sbl] = True
                    st["done"][sbl] = tbs[-1] + 1

                    def ch(sbl=sbl, tbs=tbs, out2=out2, first=first):
                        pv_mms(u, sh, sbl, out2, v_sb, tbs, first)
                    chunks.append(ch)
                return chunks

            def last_fin():
                u, sh = NU - 1, 1
                b, j = units[u]
                chunks = list(last_wave(LAST))

                def norms_a():
                    st = last_state
                    invz = sbp.tile([128, 2], f32, tag="invz", bufs=8,
                                    name="invz")
                    nc.vector.reciprocal(invz[:, 0:1], st["out2"][0][:, DH:DH + 1])
                    nc.vector.reciprocal(invz[:, 1:2], st["out2"][1][:, DH:DH + 1])
                    scale_sb(u, sh, 0, st["out2"][0][:, :DH], invz[:, 0:1], "dve")
                    scale_sb(u, sh, 1, st["out2"][1][:, :DH], invz[:, 1:2], "act")
                chunks.append(norms_a)

                def norms_b():
                    st = last_state
                    invz = sbp.tile([128, 2], f32, tag="invz", bufs=8,
                                    name="invz")
                    # sbl 2,3 live in the same ring tile: one strided recip
                    g61 = new_gen(int(CSTART[LAST - 2]) // GEN)
                    nc.vector.reciprocal(invz[:], g61[:, DH:DH + 513:512])
                    scale_sb(u, sh, 2, st["out2"][2][:, :DH], invz[:, 0:1], "dve")
                    scale_sb(u, sh, 3, st["out2"][3][:, :DH], invz[:, 1:2], "act")
                chunks.append(norms_b)

                def store_a():
                    nc.sync.dma_start(
                        out_d[b, j][:, 256:384], get_o(u)[:, 256:384]
                    )
                chunks.append(store_a)

                def store_b():
                    nc.sync.dma_start(
                        out_d[b, j][:, 384:512], get_o(u)[:, 384:512]
                    )
                chunks.append(store_b)
                return chunks

            # Software pipeline: scores pieces stream through the psum ring
            # in 512-col steps (s-major within each unit); exp fires per
            # chunk on ACT or DVE; projection chunks of the next unit and
            # PV/normalize/store chunks of completed half-units interleave
            # as fillers.
            from collections import deque

            fillers = deque()
            unit_io = {}

            def unit_inputs(u):
                b, j = units[u]
                yT, ychunks = y_chunks(b, j)
                for c in ychunks:
                    fillers.append(c)
                v_sb, vchunks = v_chunks(b, j)
                for c in vchunks:
                    fillers.append(c)
                unit_io[u] = (yT, v_sb)

            unit_inputs(0)
            u0_chunks = list(fillers)
            fillers.clear()
            for p, (g, w) in enumerate(PIECES):  # scores pieces, s-major
                u = g // 8192
                sh = (g % 8192) // 4096
                tb = (g % 4096) // 512
                so = sh * 512 + (g % 512)  # s-offset within the unit's half
                if p < len(u0_chunks):
                    u0_chunks[p]()  # y chunks before their pieces, then v
                if g % 8192 == 2048 and u + 1 < NU:
                    unit_inputs(u + 1)
                yT, _ = unit_io[u]
                if g % GEN == 0 and g // GEN not in gens:
                    new_gen(g // GEN)
                rp = g % GEN
                nc.tensor.matmul(
                    gens[g // GEN][:, rp:rp + w],
                    fetch_xt(*units[u])[:, tb * 128:(tb + 1) * 128],
                    yT[:, so:so + w],
                    start=True, stop=True,
                )
                c, _ = _chunk_of(g)
                if g + w == int(CSTART[c + 1]):  # chunk complete -> exp
                    emit_exp(c)
                    if c == LAST - 3:
                        # sbl 2,3 park in the tile chunk LAST-2 still reads
                        for ch in last_wave(c, allowed=(0, 1)):
                            fillers.append(ch)
                    elif c in (LAST - 2, LAST - 1):
                        for ch in last_wave(c):
                            fillers.append(ch)
                    elif c == LAST:
                        for ch in last_fin():
                            fillers.append(ch)
                    for hu in range(NHU - 1):
                        if C_END_H[hu] == c:
                            for ch in pv_burst(hu, unit_io[hu // 2][1]):
                                fillers.append(ch)
                for _ in range(2):
                    if fillers:
                        fillers.popleft()()
            while fillers:
                fillers.popleft()()
    if split:
        _split_sync_waits(nc)
    return nc


def _prep_inputs(sequences, Wq, Wk, Wv, bq, bk, bv):
    """Host-side packing: per-core input maps."""
    import ml_dtypes

    sequences = np.ascontiguousarray(np.asarray(sequences, dtype=np.float32))
    Wq = np.asarray(Wq, np.float32)
    Wk = np.asarray(Wk, np.float32)
    Wv = np.asarray(Wv, np.float32)
    bq = np.asarray(bq, np.float32)
    bk = np.asarray(bk, np.float32)
    bv = np.asarray(bv, np.float32)

    # [B, S, H, DH] -> [H, B, DH, S] transposed slices
    xT = np.ascontiguousarray(
        sequences.reshape(B, S, H, DH).transpose(2, 0, 3, 1)
    )  # [H, B, DH, S]

    in_maps = []
    for c in range(NCORES):
        heads = [HPC * c + j for j in range(HPC)]
        xTe = np.zeros((B, HPC, E1, S), np.float32)
        xTe[:, :, DH, :] = 1.0
        for j, h in enumerate(heads):
            xTe[:, j, :DH, :] = xT[h]
        gt = np.zeros((E1, HPC, E1), np.float32)
        wv = np.zeros((E1, HPC, E1), np.float32)
        for j, h in enumerate(heads):
            wq = np.zeros((E1, DH), np.float32)  # x~ -> q, scale folded
            wq[:DH] = Wq[h].T * SCALE
            wq[DH] = bq[h] * SCALE
            wk = np.zeros((E1, DH), np.float32)  # x~ -> k
            wk[:DH] = Wk[h].T
            wk[DH] = bk[h]
            # scores = k.q = x~^T (Wk~ Wq~^T) x~; lhsT of the y-projection
            # is the transpose: G^T = Wq~ @ Wk~^T
            gt[:, j, :] = wq @ wk.T
            wv[:DH, j, :DH] = Wv[h].T
            wv[DH, j, :DH] = bv[h]
            wv[DH, j, DH] = 1.0  # ones column -> Z column of out2
        in_maps.append({
            "xTe": xTe,
            "xbTe": xTe.astype(ml_dtypes.bfloat16),
            "GT": gt.reshape(E1, HPC * E1),
            "WvTe2": wv.reshape(E1, HPC * E1).astype(ml_dtypes.bfloat16),
        })
    return in_maps


def get_nc():
    if "nc" not in _CACHE:
        _CACHE["nc"] = _build_bass()
    return _CACHE["nc"]


def kernel(sequences, Wq, Wk, Wv, bq, bk, bv):
    from concourse.bass_utils import run_bass_kernel_spmd

    nc = get_nc()
    in_maps = _prep_inputs(sequences, Wq, Wk, Wv, bq, bk, bv)
    res = run_bass_kernel_spmd(nc, in_maps, list(range(NCORES)))
    full = np.empty((B, S, D), np.float32)
    for c in range(NCORES):
        # out[b, j, p, blk*64+e] -> full[b, blk*128+p, (2c+j)*64+e]
        arr = res.results[c]["out"].reshape(B, HPC, 128, NT, DH)
        full[:, :, c * HPC * DH:(c + 1) * HPC * DH] = (
            arr.transpose(0, 3, 2, 1, 4).reshape(B, S, HPC * DH)
        )
    return full
